# revision 33
# baseline (speedup 1.0000x reference)
"""DeltaNet-style block (nn_DeltaNet_31877247271438) on 8 trn2 NeuronCores.

Sharding: core c -> (batch b = c//2, pair-rank r = c%2).  Within a batch pair:
  - head-parallel: rank r owns heads {2r, 2r+1} (feature cols [512r, 512r+512))
  - cross-head mixes are K-split with pairwise collectives:
      * channel_mixer (folded with kernel_mix into one matrix Q): partial sums
        ReduceScatter'ed (f16) so each core receives its own heads' ms_out
      * fusion-MLP hidden is column-split; logits partials ReduceScatter'ed
      * bn features AllGather'ed (tiny)
  - the final Wo matmul partials are summed on the host.

v2: single-pass schedule tuned for the TRN2 cost model:
  - all activations/projections stay SBUF-resident (no DRAM staging except
    the collectives and delta outputs); DMA count cut ~10x
  - delta rule runs as two 8-chunk batched halves (stage-major issue keeps
    the PE stream dense) followed by a short sequential scan over chunks;
    beta is folded in via per-partition scalar ops so no kb tensor exists
  - PSUM->SBUF copies round-robin over Act/DVE/Pool
  - fusion-MLP hidden (hs @ W1) is precomputed before the delta rule so only
    the small bn-correction + gelu remain after the AllGather
"""
import sys
sys.path.insert(0, '/opt/trn_rl_repo')

import numpy as np
import ml_dtypes

import concourse.bass as bass
import concourse.tile as tile
from concourse import bacc, mybir
from concourse.bass_utils import run_bass_kernel_spmd

F32 = mybir.dt.float32
BF16 = mybir.dt.bfloat16
F16 = mybir.dt.float16
F8 = mybir.dt.float8e4
AF = mybir.ActivationFunctionType
ALU = mybir.AluOpType

B, L, D, H = 4, 2048, 1024, 4
d = 256          # per-head dim
C = 512          # channels owned per core (2 heads)
NLT = 16         # l-tiles of 128
NLW = 4          # l-windows of 512
NCH = 16         # delta chunks of 128
KQKV = 4         # qkv conv taps
MSK = (3, 15, 31)
NTAPS = sum(MSK)  # 49
MSP = (2, 8, 16)  # fp8 DoubleRow tap-pairs per scale (taps padded to even)
NPAIRS = sum(MSP)  # 26
MS_SCALE = 32.0
QMIX_SCALE = 32.0
CM_DESCALE = 1.0 / (MS_SCALE * QMIX_SCALE)
PADV = 32
PAD = 4
RG = [[0, 1], [2, 3], [4, 5], [6, 7]]


def bc_mid(ap2, n):
    """[P, F] AP -> [P, n, F] with a 0-stride middle dim (free-dim bcast)."""
    assert len(ap2.ap) == 2
    return bass.AP(tensor=ap2.tensor, offset=ap2.offset,
                   ap=[ap2.ap[0], [0, n], ap2.ap[1]])


def build_program(debug=False):
    nc = bacc.Bacc("TRN2", target_bir_lowering=False, debug=False,
                   num_devices=8)

    io = {}
    io["hsT"] = nc.declare_dram_parameter("hsT", [D, L], F16, False)
    io["wq"] = nc.declare_dram_parameter("wq", [D, C], F16, False)
    io["wk"] = nc.declare_dram_parameter("wk", [D, C], F16, False)
    io["wv"] = nc.declare_dram_parameter("wv", [D, C], F16, False)
    io["wb"] = nc.declare_dram_parameter("wb", [D, 2], F16, False)
    io["cdiag"] = nc.declare_dram_parameter("cdiag", [3, 128, 4, KQKV, 128],
                                            F16, False)
    io["msdiag"] = nc.declare_dram_parameter("msdiag", [4, 128, NPAIRS, 2, 128],
                                             F8, False)
    io["qmix"] = nc.declare_dram_parameter("qmix", [12 * 128, D], F8, False)
    io["fw1h"] = nc.declare_dram_parameter("fw1h", [D, 1024], F16, False)
    io["fw1b"] = nc.declare_dram_parameter("fw1b", [16, 1024], F16, False)
    io["fb1"] = nc.declare_dram_parameter("fb1", [1024], F32, False)
    io["fw2"] = nc.declare_dram_parameter("fw2", [1024, 12], F16, False)
    io["b2o"] = nc.declare_dram_parameter("b2o", [128, 6], F32, False)
    io["wo"] = nc.declare_dram_parameter("wo", [C, D], F16, False)
    io["masks"] = nc.declare_dram_parameter("masks", [5, 128, 128], F32, False)
    io["onesrow"] = nc.declare_dram_parameter("onesrow", [1, 128], F32, False)
    io["onescol"] = nc.declare_dram_parameter("onescol", [128, 1], F32, False)
    io["ident16"] = nc.declare_dram_parameter("ident16", [128, 128], F16, False)
    io["out_part"] = nc.declare_dram_parameter("out_part", [L, D], F32, True)

    sc = {}
    sc["qT_s"] = nc.dram_tensor("qT_s", [2, 2, 128, L], F16)
    sc["hdn_s"] = nc.dram_tensor("hdn_s", [1024, L], F16)
    sc["cm_in"] = nc.dram_tensor("cm_in", [2, L, C], F16)
    sc["cm_out"] = nc.dram_tensor("cm_out", [L, C], F16)
    sc["dout_s"] = nc.dram_tensor("dout_s", [L, C], F16)
    sc["bn_in"] = nc.dram_tensor("bn_in", [L, 8], F32)
    sc["bn_out"] = nc.dram_tensor("bn_out", [2, L, 8], F32)
    sc["lg_in"] = nc.dram_tensor("lg_in", [2, 2, L // 2, 6], F32)
    sc["lg_out"] = nc.dram_tensor("lg_out", [2, L // 2, 6], F32)

    with tile.TileContext(nc) as tc:
        _body(nc, tc, io, sc)
    nc.compile()
    return nc


def _body(nc, tc, io, sc):
    from contextlib import ExitStack
    ctx = ExitStack()
    with ctx:
        consts = ctx.enter_context(tc.tile_pool(name="consts", bufs=1))
        glob = ctx.enter_context(tc.tile_pool(name="glob", bufs=1))

        masks = consts.tile([128, 5, 128], F32)
        nc.sync.dma_start(out=masks,
                          in_=io["masks"][:].rearrange("m p f -> p m f"))
        ident = masks[:, 4, :]
        ident16 = consts.tile([128, 128], F16)
        nc.sync.dma_start(out=ident16, in_=io["ident16"][:])
        onescol16 = consts.tile([128, 1], F16)
        nc.vector.memset(onescol16, 1.0)
        onesrow16 = consts.tile([1, 128], F16)
        nc.vector.memset(onesrow16, 1.0)
        eps6 = consts.tile([128, 1], F32)
        nc.vector.memset(eps6, 1e-6)
        eps5 = consts.tile([128, 1], F32)
        nc.vector.memset(eps5, 1e-5)

        beta_lp = consts.tile([128, NLT, 2], F32)
        # delta-rule state, ping-pong buffered across scan chunks
        S16 = consts.tile([128, 2, 2, 2, d], F16)  # (buf, h, kt, dv)
        nc.vector.memset(S16, 0.0)

        # persistent activations
        kT_sb = glob.tile([128, 2, 2, L], F16)     # k-hat, (h, dk-tile, l)
        klc = glob.tile([128, NLT, 2, d], F16)     # k-hat, l-major
        vlc = glob.tile([128, NLT, 2, d], F16)     # v, l-major
        bn_sb = glob.tile([128, NLT, 8], F32)
        nc.vector.memset(bn_sb, 0.0)

        hsT_r = io["hsT"][:].rearrange("(kt p) l -> p kt l", p=128)

        def cp(i, out, in_):
            # Pool/GPSIMD cannot touch PSUM; alternate Act and DVE
            if i % 2 == 0:
                nc.scalar.copy(out=out, in_=in_)
            else:
                nc.vector.tensor_copy(out=out, in_=in_)

        pvt_cm = tc.tile_pool(name="pvt", bufs=1)
        pvt = pvt_cm.__enter__()
        vt_bf = pvt.tile([128, 4, PADV + L], F16)  # conv-input v
        nc.vector.memset(vt_bf[:, :, 0:PADV], 0.0)
        v8_bf = pvt.tile([128, 4, PADV + L], F8)   # fp8 copy for ms-conv
        nc.gpsimd.memset(v8_bf[:, :, 0:PADV], 0.0)

        # =================== PHASE A ======================================
        with tc.tile_pool(name="pa1", bufs=1) as pa1, \
             tc.tile_pool(name="pa2", bufs=2) as pa2, \
             tc.tile_pool(name="pas", bufs=3) as pas, \
             tc.tile_pool(name="psa", bufs=4, space="PSUM") as psa, \
             tc.tile_pool(name="psb", bufs=2, space="PSUM") as psb:
            hsT = pa1.tile([128, 8, L], F16)
            nc.sync.dma_start(out=hsT, in_=hsT_r)

            def ps(name="pst", tag="pst"):
                return psa.tile([128, 512], F32, tag=tag, name=name)

            def ps16(name="pst16"):
                return psb.tile([128, 512], F16, tag="ps16", name=name)

            # ---- beta (row-form matmuls, then transpose to column) -------
            wb_sb = pa1.tile([128, 8, 2], F16)
            nc.sync.dma_start(
                out=wb_sb, in_=io["wb"][:].rearrange("(kt p) c -> p kt c",
                                                     p=128))
            for lw in range(NLW):
                pb = ps("psbeta")
                pbv = pb[0:2, :]
                for kt in range(8):
                    nc.tensor.matmul(pbv, wb_sb[:, kt, :],
                                     hsT[:, kt, lw*512:(lw+1)*512],
                                     start=(kt == 0), stop=(kt == 7))
                betar = pas.tile([2, 512], F32, tag="betar", bufs=2,
                                 name="betar")
                nc.scalar.activation(out=betar, in_=pbv, func=AF.Sigmoid)
                for ltl in range(4):
                    pt = ps("psbt")
                    ptv = pt[0:128, 0:2]
                    nc.tensor.transpose(ptv, betar[:, ltl*128:(ltl+1)*128],
                                        ident[0:2, 0:2])
                    nc.vector.tensor_copy(out=beta_lp[:, lw*4+ltl, :],
                                          in_=ptv)

            fw1h_sb = pa1.tile([128, 8, 1024], F16)
            nc.sync.dma_start(
                out=fw1h_sb,
                in_=io["fw1h"][:].rearrange("(kt p) m -> p kt m", p=128))

            # ---- q, k, v: proj -> conv -> silu ---------------------------
            # v first (feeds ms-conv + vlc); k silus straight into kT_sb
            # (normalized in place later); q stages through xc then DRAM.
            xc = None
            for tnm, ti in (("v", 2), ("q", 0), ("k", 1)):
                w_sb = pa2.tile([128, 8, C], F16, tag="w_sb", bufs=2,
                                name=f"w_{tnm}")
                nc.sync.dma_start(
                    out=w_sb,
                    in_=io["w" + tnm][:].rearrange("(kt p) c -> p kt c",
                                                   p=128))
                cdg = pa2.tile([128, 4, KQKV, 128], F16, tag="cdg", bufs=1,
                               name=f"cdg_{tnm}")
                nc.sync.dma_start(out=cdg, in_=io["cdiag"][ti])
                if tnm == "q":
                    xc = pa1.tile([128, 4, PAD + L], F16, name="xc_q")
                    nc.gpsimd.memset(xc[:, :, 0:PAD], 0.0)

                for ct in range(4):
                    xp = pa2.tile([128, PAD + L], F16, tag="xp", bufs=2,
                                  name=f"xp_{tnm}{ct}")
                    nc.gpsimd.memset(xp[:, 0:PAD], 0.0)
                    for lw in range(NLW):
                        pp = ps("psp")
                        for kt in range(8):
                            nc.tensor.matmul(
                                pp, w_sb[:, kt, ct*128:(ct+1)*128],
                                hsT[:, kt, lw*512:(lw+1)*512],
                                start=(kt == 0), stop=(kt == 7))
                        cp(ct + lw, xp[:, PAD+lw*512:PAD+(lw+1)*512], pp)
                    for lw in range(NLW):
                        pc = ps("psc")
                        for dd in range(KQKV):
                            off = PAD + lw*512 - dd
                            nc.tensor.matmul(
                                pc, cdg[:, ct, dd, :],
                                xp[:, off:off+512],
                                start=(dd == 0), stop=(dd == KQKV-1))
                        lsl = slice(lw*512, (lw+1)*512)
                        if tnm == "v":
                            nc.scalar.activation(
                                out=vt_bf[:, ct, PADV+lw*512:PADV+(lw+1)*512],
                                in_=pc, func=AF.Silu)
                        elif tnm == "k":
                            nc.scalar.activation(
                                out=kT_sb[:, ct // 2, ct % 2, lsl],
                                in_=pc, func=AF.Silu)
                        else:
                            nc.scalar.activation(
                                out=xc[:, ct, PAD+lw*512:PAD+(lw+1)*512],
                                in_=pc, func=AF.Silu)
                if tnm == "v":
                    # transpose v into l-major vlc
                    for ct in range(4):
                        h, dt = ct // 2, ct % 2
                        for lt in range(NLT):
                            ptr = ps16("psvt")
                            ptv = ptr[:, 0:128]
                            nc.tensor.transpose(
                                ptv,
                                vt_bf[:, ct, PADV+lt*128:PADV+(lt+1)*128],
                                ident16)
                            cp(lt % 3 % 2, vlc[:, lt, h, dt*128:(dt+1)*128], ptv)
                    for lt in range(NLT):
                        nc.vector.tensor_reduce(
                            out=bn_sb[:, lt, 4:6], in_=vlc[:, lt],
                            axis=mybir.AxisListType.X, op=ALU.add,
                            apply_absolute_value=True)
                    for ct in range(4):
                        nc.gpsimd.tensor_copy(out=v8_bf[:, ct, PADV:],
                                              in_=vt_bf[:, ct, PADV:])

            # ---- l2norm (q -> DRAM, k in place) + klc transposes,
            # interleaved with the fusion-MLP hidden matmuls so the PE
            # stream stays dense while the norm chains hop engines.
            filler = []

            def norm_item(tj, tnm, h, lw):
                def emit():
                    lsl = slice(lw*512, (lw+1)*512)

                    def src(dt):
                        if tnm == "q":
                            return xc[:, 2*h+dt, PAD+lsl.start:PAD+lsl.stop]
                        return kT_sb[:, h, dt, lsl]

                    sq = pas.tile([128, 2, 512], F16, tag="sq", bufs=2,
                                  name="sq")
                    pss = ps("psss")
                    pssv = pss[0:1, :]
                    for i in range(2):
                        nc.gpsimd.tensor_tensor(
                            out=sq[:, i, :], in0=src(i), in1=src(i),
                            op=ALU.mult)
                        nc.tensor.matmul(pssv, onescol16, sq[:, i, :],
                                         start=(i == 0), stop=(i == 1))
                    sr = pas.tile([1, 512], F32, tag="sr", bufs=2, name="sr")
                    nc.scalar.activation(out=sr, in_=pssv, func=AF.Sqrt,
                                         bias=eps6[0:1, :])
                    srt = pas.tile([1, 512], F16, tag="srt", bufs=2,
                                   name="srt")
                    with nc.allow_low_precision("l2norm scale fp16"):
                        nc.vector.reciprocal(out=srt, in_=sr)
                    pbc = ps("psbc")
                    nc.tensor.matmul(pbc, onesrow16, srt,
                                     start=True, stop=True)
                    for dt in range(2):
                        if tnm == "q":
                            qn = pas.tile([128, 512], F16, tag="qn",
                                          bufs=3, name="qn")
                            nc.vector.tensor_tensor(
                                out=qn, in0=src(dt), in1=pbc, op=ALU.mult)
                            nc.sync.dma_start(
                                out=sc["qT_s"][h, dt, :, lsl], in_=qn)
                        else:
                            nc.vector.tensor_tensor(
                                out=kT_sb[:, h, dt, lsl],
                                in0=src(dt), in1=pbc, op=ALU.mult)
                return emit

            def ktr_item(ct, lt0):
                def emit():
                    h, dt = ct // 2, ct % 2
                    for lt in range(lt0, lt0 + 8):
                        ptr = ps16("pskt")
                        ptv = ptr[:, 0:128]
                        nc.tensor.transpose(
                            ptv, kT_sb[:, h, dt, lt*128:(lt+1)*128],
                            ident16)
                        cp((lt + 1) % 3 % 2,
                           klc[:, lt, h, dt*128:(dt+1)*128], ptv)
                return emit

            for tnm, tj in (("q", 0), ("k", 1)):
                for h in range(2):
                    for lw in range(NLW):
                        filler.append(norm_item(tj, tnm, h, lw))
            for ct in range(4):
                for lt0 in (0, 8):
                    filler.append(ktr_item(ct, lt0))

            # fusion-MLP hidden partial (hs @ W1), pre-gelu -> DRAM
            for lw in range(NLW):
                hdl = pas.tile([128, 8, 512], F16, tag="hdl", bufs=2,
                               name="hdl")
                for mt in range(8):
                    ph = ps("psh")
                    for kt in range(8):
                        nc.tensor.matmul(ph,
                                         fw1h_sb[:, kt, mt*128:(mt+1)*128],
                                         hsT[:, kt, lw*512:(lw+1)*512],
                                         start=(kt == 0), stop=(kt == 7))
                    cp(lw + mt, hdl[:, mt, :], ph)
                    if filler:
                        filler.pop(0)()
                nc.sync.dma_start(
                    out=sc["hdn_s"][:, lw*512:(lw+1)*512].rearrange(
                        "(mt p) l -> p mt l", p=128),
                    in_=hdl)
            while filler:
                filler.pop(0)()

        # =================== PHASE B1/B2 (fused per l-window) =============
        # ms-conv + qmix run in fp8 DoubleRow: two shifted taps (or two
        # k-tiles) contract per instruction at 0.5 cycles/row.
        DR = mybir.MatmulPerfMode.DoubleRow
        with tc.tile_pool(name="pb1", bufs=1) as pb1, \
             tc.tile_pool(name="pbs", bufs=2) as pbs, \
             tc.tile_pool(name="psB", bufs=6, space="PSUM") as psB:
            qmix_sb = pb1.tile([128, 12, D], F8)
            nc.sync.dma_start(
                out=qmix_sb,
                in_=io["qmix"][:].rearrange("(kt p) o -> p kt o", p=128))
            ms8 = pb1.tile([128, 4, NPAIRS, 2, 128], F8)
            for ct in range(4):
                nc.sync.dma_start(out=ms8[:, ct], in_=io["msdiag"][ct])

            def pair_rhs(ct, off):
                # [128, 2, 512] with +1 stride: slice i starts at off-1+i,
                # i.e. i=0 -> tap 2t+1, i=1 -> tap 2t (host packs to match)
                a = v8_bf[:, ct, off-1:off-1+512]
                return bass.AP(tensor=a.tensor, offset=a.offset,
                               ap=[a.ap[0], [1, 2], a.ap[1]])

            for lw in range(NLW):
                y8 = pbs.tile([128, 12, 512], F8, tag="y", bufs=1, name="y")
                for ct in range(4):
                    base = 0
                    for si, np_ in enumerate(MSP):
                        py = psB.tile([128, 512], F32, tag="pB", name="psy")
                        for t in range(np_):
                            off = PADV + lw*512 - 2*t
                            nc.tensor.matmul(py, ms8[:, ct, base+t],
                                             pair_rhs(ct, off),
                                             start=(t == 0),
                                             stop=(t == np_-1),
                                             perf_mode=DR)
                        cp(ct + si, y8[:, si*4+ct, :], py)
                        base += np_
                cm_sb = pbs.tile([128, 4, 2, 512], F16, tag="cm",
                                 name=f"cm{lw}")
                for ltl in range(4):
                    for oh in range(2):
                        pq = psB.tile([128, 512], F32, tag="pB", name="psq")
                        for kp in range(6):
                            nc.tensor.matmul(
                                pq, y8[:, 2*kp:2*kp+2, ltl*128:(ltl+1)*128],
                                qmix_sb[:, 2*kp:2*kp+2, oh*512:(oh+1)*512],
                                start=(kp == 0), stop=(kp == 5),
                                perf_mode=DR)
                        if (ltl + oh) % 2 == 0:
                            nc.scalar.mul(out=cm_sb[:, ltl, oh, :], in_=pq,
                                          mul=CM_DESCALE)
                        else:
                            nc.vector.tensor_scalar_mul(
                                cm_sb[:, ltl, oh, :], pq, CM_DESCALE)
                for oh in range(2):
                    nc.sync.dma_start(
                        out=sc["cm_in"][oh, lw*512:(lw+1)*512, :].rearrange(
                            "(lt p) c -> p lt c", p=128),
                        in_=cm_sb[:, :, oh, :])
            nc.gpsimd.collective_compute(
                "ReduceScatter", ALU.add, replica_groups=RG,
                ins=[sc["cm_in"][:]], outs=[sc["cm_out"][:]])
        pvt_cm.__exit__(None, None, None)

        # =================== PHASE B3: delta rule =========================
        HC = 4                    # chunks per batch
        NB = NCH // HC            # number of batches
        NW = 2                    # 2-chunk waves per batch

        def bcm(m, n1=2, n2=2):
            """masks16[:, m, :] broadcast to [128, n1, n2, 128]."""
            ap = masks16[:, m, :]
            return bass.AP(tensor=ap.tensor, offset=ap.offset,
                           ap=[ap.ap[0], [0, n1], [0, n2], ap.ap[1]])

        ones128 = consts.tile([128, 128], F16)
        nc.vector.memset(ones128, 1.0)
        masks16 = consts.tile([128, 5, 128], F16)
        nc.vector.tensor_copy(out=masks16, in_=masks)

        with tc.tile_pool(name="pd1", bufs=1) as pd1, \
             tc.tile_pool(name="pdh", bufs=2) as pdh, \
             tc.tile_pool(name="pdo", bufs=3) as pdo, \
             tc.tile_pool(name="psD", bufs=4, space="PSUM") as psD, \
             tc.tile_pool(name="psU", bufs=3, space="PSUM") as psU:
            cmv = pd1.tile([128, NLT, 2, d], F16)

            def pd_ps(name):
                # one 2-chunk wave of 128-wide mats (2KB)
                return psD.tile([128, 2, 2, 128], F32, tag="pD", name=name)

            def pd16_ps(name, shape=None):
                return psD.tile(shape or [128, 2, 2, 128], F16, tag="pD",
                                name=name)

            def pw_ps(name):
                # one chunk of d-wide mats (2KB)
                return psD.tile([128, 2, d], F32, tag="pD", name=name)

            def pu_ps(name):
                return psU.tile([128, 2, d], F32, tag="pU", name=name)

            def hblk(tag, name=None):
                return pdh.tile([128, HC, 2, 128], F16, tag=tag,
                                name=name or tag)

            cp_i = [0]

            def bigcp(out, in_):
                cp_i[0] += 1
                cp(0 if cp_i[0] % 3 < 2 else 1, out, in_)

            def psum_add(i, out, psum, addend):
                """out = psum + addend; odd i offloads to Act+Pool."""
                if i % 2 == 0:
                    nc.vector.scalar_tensor_tensor(
                        out=out, in0=psum, scalar=1.0, in1=addend,
                        op0=ALU.mult, op1=ALU.add)
                else:
                    nc.scalar.copy(out=out, in_=psum)
                    nc.gpsimd.tensor_tensor(out=out, in0=out, in1=addend,
                                            op=ALU.add)

            def scan_chunk(qTb, u_all, wT_all, aT_all, j, ci):
                prev = ci % 2
                cur = 1 - prev
                pup = pu_ps(f"pup{ci}")
                for h in range(2):
                    for kt in range(2):
                        nc.tensor.matmul(pup[:, h, :],
                                         wT_all[:, j, kt, h, :],
                                         S16[:, prev, h, kt, :],
                                         start=(kt == 0), stop=(kt == 1))
                upr = pdo.tile([128, 2, d], F16, tag="upr", name="upr")
                nc.vector.scalar_tensor_tensor(
                    out=upr, in0=pup, scalar=-1.0, in1=u_all[:, j],
                    op0=ALU.mult, op1=ALU.add)
                po = pu_ps(f"po{ci}")
                for h in range(2):
                    for kt in range(2):
                        nc.tensor.matmul(po[:, h, :],
                                         qTb[:, h, kt, j*128:(j+1)*128],
                                         S16[:, prev, h, kt, :],
                                         start=(kt == 0), stop=False)
                    nc.tensor.matmul(po[:, h, :], aT_all[:, j, h, :],
                                     upr[:, h, :], start=False, stop=True)
                for h in range(2):
                    pdS = pu_ps(f"pdS{ci}{h}")
                    for kt in range(2):
                        nc.tensor.matmul(pdS[:, kt, :],
                                         klc[:, ci, h, kt*128:(kt+1)*128],
                                         upr[:, h, :],
                                         start=True, stop=True)
                    nc.vector.scalar_tensor_tensor(
                        out=S16[:, cur, h], in0=pdS, scalar=1.0,
                        in1=S16[:, prev, h], op0=ALU.mult, op1=ALU.add)
                ot = pdo.tile([128, 2, d], F16, tag="ot", name="ot")
                nc.scalar.copy(out=ot, in_=po)
                babs = pdo.tile([128, 2, d], F16, tag="babs", name="babs")
                for h in range(2):
                    nc.scalar.activation(out=babs[:, h], in_=ot[:, h],
                                         func=AF.Abs,
                                         accum_out=bn_sb[:, ci, 2+h:3+h])
                nc.sync.dma_start(
                    out=sc["dout_s"][ci*128:(ci+1)*128, :],
                    in_=ot.rearrange("p h e -> p (h e)"))

            results = {}

            def batch_stages(bi):
                cs = list(range(bi*HC, (bi+1)*HC))
                qTb = pdh.tile([128, 2, 2, HC * 128], F16, tag="qTb",
                               bufs=4, name=f"qTb{bi}")
                nc.sync.dma_start(
                    out=qTb,
                    in_=sc["qT_s"][:, :, :,
                                   cs[0]*128:(cs[-1]+1)*128].rearrange(
                        "h kt p l -> p h kt l"))
                bB = pdh.tile([128, HC, 2, 128], F16, tag="tBB", name="bB")
                for j, ci in enumerate(cs):
                    for h in range(2):
                        nc.gpsimd.tensor_scalar_mul(
                            bB[:, j, h, :], ones128,
                            beta_lp[:, ci, h:h+1])
                yield

                Td, To, TdT = hblk("tA", "Td"), hblk("tTo", "To"), \
                    hblk("tB", "TdT")
                aT = pdh.tile([128, HC, 2, 128], F16, tag="taT", bufs=4,
                              name="aT")
                for wv in range(NW):
                    pg = pd_ps(f"pg{wv}")
                    pa_ = pd_ps(f"pa{wv}")
                    jsl = slice(wv*2, (wv+1)*2)
                    for jj in range(2):
                        j = wv*2 + jj
                        ci = cs[j]
                        for h in range(2):
                            for kt in range(2):
                                nc.tensor.matmul(
                                    pg[:, jj, h, :],
                                    kT_sb[:, h, kt, ci*128:(ci+1)*128],
                                    kT_sb[:, h, kt, ci*128:(ci+1)*128],
                                    start=(kt == 0), stop=(kt == 1))
                            for kt in range(2):
                                nc.tensor.matmul(
                                    pa_[:, jj, h, :],
                                    kT_sb[:, h, kt, ci*128:(ci+1)*128],
                                    qTb[:, h, kt, j*128:(j+1)*128],
                                    start=(kt == 0), stop=(kt == 1))
                    bG = pdo.tile([128, 2, 2, 128], F16, tag="bG",
                                  name=f"bG{wv}")
                    nc.vector.tensor_tensor(out=bG, in0=pg,
                                            in1=bB[:, jsl], op=ALU.mult)
                    nc.vector.tensor_tensor(out=Td[:, jsl], in0=bG,
                                              in1=bcm(0), op=ALU.mult)
                    nc.gpsimd.tensor_tensor(out=To[:, jsl], in0=bG,
                                            in1=bcm(1), op=ALU.mult)
                    nc.vector.tensor_tensor(out=aT[:, jsl], in0=pa_,
                                            in1=bcm(3), op=ALU.mult)
                    yield

                for wv in range(NW):
                    jsl = slice(wv*2, (wv+1)*2)
                    ptr = pd16_ps(f"ptt{wv}")
                    for jj in range(2):
                        j = wv*2 + jj
                        for h in range(2):
                            nc.tensor.transpose(ptr[:, jj, h, :],
                                                Td[:, j, h, :], ident16)
                    bigcp(TdT[:, jsl], ptr)
                yield
                MT = hblk("tE", "MT0")
                nc.vector.tensor_tensor(out=MT, in0=TdT,
                                        in1=bcm(4, n1=HC), op=ALU.add)
                lvl = {"Td": Td, "TdT": TdT}
                chain = [
                    ("T2", "TdT", "Td", "tC"), ("T2T", "Td", "TdT", "tD"),
                    ("T4", "T2T", "T2", "tA"), ("T4T", "T2", "T2T", "tB"),
                    ("T8", "T4T", "T4", "tC"), ("T8T", "T4", "T4T", "tD"),
                    ("T16", "T8T", "T8", "tA"),
                ]
                mt_tags = {"T2": "tF", "T4": "tE", "T8": "tF", "T16": "tG"}
                for si, (nm, ln, rn, tg) in enumerate(chain):
                    dst = hblk(tg, nm)
                    for wv in range(NW):
                        jsl = slice(wv*2, (wv+1)*2)
                        pq2 = pd_ps(f"pq_{nm}{wv}")
                        for jj in range(2):
                            j = wv*2 + jj
                            for h in range(2):
                                nc.tensor.matmul(
                                    pq2[:, jj, h, :], lvl[ln][:, j, h, :],
                                    lvl[rn][:, j, h, :],
                                    start=True, stop=True)
                        bigcp(dst[:, jsl], pq2)
                    lvl[nm] = dst
                    yield
                    if nm in mt_tags:
                        MTn = pdh.tile([128, HC, 2, 128], F16,
                                       tag=mt_tags[nm], name=f"MT_{nm}")
                        for wv in range(NW):
                            jsl = slice(wv*2, (wv+1)*2)
                            pm = pd_ps(f"pm_{nm}{wv}")
                            for jj in range(2):
                                j = wv*2 + jj
                                for h in range(2):
                                    nc.tensor.matmul(pm[:, jj, h, :],
                                                     lvl[nm][:, j, h, :],
                                                     MT[:, j, h, :],
                                                     start=True, stop=True)
                            psum_add(si + wv, MTn[:, jsl], pm, MT[:, jsl])
                        MT = MTn
                        yield
                DT = MT

                Bm, BT = hblk("tA", "Bm"), hblk("tB", "BT")
                for wv in range(NW):
                    jsl = slice(wv*2, (wv+1)*2)
                    pB = pd_ps(f"pB{wv}")
                    pBT = pd_ps(f"pBT{wv}")
                    for jj in range(2):
                        j = wv*2 + jj
                        for h in range(2):
                            nc.tensor.matmul(pB[:, jj, h, :],
                                             DT[:, j, h, :],
                                             To[:, j, h, :],
                                             start=True, stop=True)
                            nc.tensor.matmul(pBT[:, jj, h, :],
                                             To[:, j, h, :],
                                             DT[:, j, h, :],
                                             start=True, stop=True)
                    bigcp(Bm[:, jsl], pB)
                    bigcp(BT[:, jsl], pBT)
                yield
                B2T = hblk("tC", "B2T")
                for wv in range(NW):
                    jsl = slice(wv*2, (wv+1)*2)
                    pB2 = pd_ps(f"pB2{wv}")
                    for jj in range(2):
                        j = wv*2 + jj
                        for h in range(2):
                            nc.tensor.matmul(pB2[:, jj, h, :],
                                             Bm[:, j, h, :],
                                             BT[:, j, h, :],
                                             start=True, stop=True)
                    bigcp(B2T[:, jsl], pB2)
                yield

                u_all = pdh.tile([128, HC, 2, d], F16, tag="tU", name="u")
                w_all = pdh.tile([128, HC, 2, d], F16, tag="tW", name="w")
                for xi, (xnm, srcv, dstt) in enumerate(
                        (("u", vlc, u_all), ("w", klc, w_all))):
                    xb = pdh.tile([128, HC, 2, d], F16, tag="tXB", name="xb")
                    for j, ci in enumerate(cs):
                        for h in range(2):
                            nc.gpsimd.tensor_scalar_mul(
                                xb[:, j, h, :], srcv[:, ci, h, :],
                                beta_lp[:, ci, h:h+1])
                    x1 = pdh.tile([128, HC, 2, d], F16, tag="tX1", name="x1")
                    for j in range(HC):
                        px1 = pw_ps(f"px1{xnm}{j}")
                        for h in range(2):
                            nc.tensor.matmul(px1[:, h, :], DT[:, j, h, :],
                                             xb[:, j, h, :],
                                             start=True, stop=True)
                        bigcp(x1[:, j], px1)
                    yield
                    y1 = pdh.tile([128, HC, 2, d], F16, tag="tY1", name="y1")
                    for j in range(HC):
                        py1 = pw_ps(f"py1{xnm}{j}")
                        for h in range(2):
                            nc.tensor.matmul(py1[:, h, :], B2T[:, j, h, :],
                                             x1[:, j, h, :],
                                             start=True, stop=True)
                        psum_add(j + xi, y1[:, j], py1, x1[:, j])
                    yield
                    for j in range(HC):
                        pu = pw_ps(f"pu{xnm}{j}")
                        for h in range(2):
                            nc.tensor.matmul(pu[:, h, :], BT[:, j, h, :],
                                             y1[:, j, h, :],
                                             start=True, stop=True)
                        psum_add(j + xi + 1, dstt[:, j], pu, y1[:, j])
                    yield

                wT_all = pdh.tile([128, HC, 2, 2, 128], F16, tag="tWT",
                                  name="wT")
                for wv in range(NW):
                    ptw = pd16_ps(f"ptw{wv}", [128, 2, 2, 2, 128])
                    for jj in range(2):
                        j = wv*2 + jj
                        for kt in range(2):
                            for h in range(2):
                                nc.tensor.transpose(
                                    ptw[:, jj, kt, h, :],
                                    w_all[:, j, h, kt*128:(kt+1)*128],
                                    ident16)
                    bigcp(wT_all[:, wv*2:(wv+1)*2], ptw)
                yield
                results[bi] = (qTb, u_all, wT_all, aT)

            # drive batches pairwise; scans of the previous pair interleave
            for pair in range(NB // 2):
                if pair == 1:
                    # RS is done (or nearly) by now; safe to queue this load
                    nc.sync.dma_start(
                        out=cmv,
                        in_=sc["cm_out"][:].rearrange(
                            "(lt p) (h e) -> p lt h e", p=128, e=d))
                gens = [batch_stages(2*pair), batch_stages(2*pair + 1)]
                scan_q = ([(2*(pair-1)*HC + j) for j in range(2*HC)]
                          if pair > 0 else [])
                live = list(gens)
                while live:
                    for g in list(live):
                        try:
                            next(g)
                        except StopIteration:
                            live.remove(g)
                    if scan_q:
                        ci = scan_q.pop(0)
                        bsrc = results[ci // HC]
                        scan_chunk(bsrc[0], bsrc[1], bsrc[2], bsrc[3],
                                   ci % HC, ci)
                while scan_q:
                    ci = scan_q.pop(0)
                    bsrc = results[ci // HC]
                    scan_chunk(bsrc[0], bsrc[1], bsrc[2], bsrc[3],
                               ci % HC, ci)
            for ci in range(2*HC, NCH):
                bsrc = results[ci // HC]
                scan_chunk(bsrc[0], bsrc[1], bsrc[2], bsrc[3],
                           ci % HC, ci)
                for lt in (2*(ci - 2*HC), 2*(ci - 2*HC) + 1):
                    nc.vector.tensor_reduce(
                        out=bn_sb[:, lt, 0:2], in_=cmv[:, lt],
                        axis=mybir.AxisListType.X, op=ALU.add,
                        apply_absolute_value=True)
            nc.sync.dma_start(
                out=sc["bn_in"][:].rearrange("(lt p) c -> p lt c", p=128),
                in_=bn_sb)
            nc.gpsimd.collective_compute(
                "AllGather", ALU.bypass, replica_groups=RG,
                ins=[sc["bn_in"][:]], outs=[sc["bn_out"][:]])

        # =================== PHASE C ======================================
        with tc.tile_pool(name="pc1", bufs=1) as pc1, \
             tc.tile_pool(name="pc2", bufs=2) as pc2, \
             tc.tile_pool(name="pcs", bufs=3) as pcs, \
             tc.tile_pool(name="psC", bufs=4, space="PSUM") as psC, \
             tc.tile_pool(name="psc2", bufs=2, space="PSUM") as psc2:

            def ps(name="pst"):
                return psC.tile([128, 512], F32, tag="pC", name=name)

            def ps16(name="pst16"):
                return psc2.tile([128, 512], F16, tag="pc16", name=name)

            # ---- bn features (cm reduced + AG'd at end of B3) ------------
            cmv = pc1.tile([128, NLT, 2, d], F16)
            nc.sync.dma_start(
                out=cmv,
                in_=sc["cm_out"][:].rearrange("(lt p) (h e) -> p lt h e",
                                              p=128, e=d))
            dov = pc1.tile([128, NLT, 2, d], F16)
            nc.sync.dma_start(
                out=dov,
                in_=sc["dout_s"][:].rearrange("(lt p) (h e) -> p lt h e",
                                              p=128, e=d))

            bnT = [pc1.tile([8, L], F16, name=f"bnT{m}") for m in range(2)]
            for m in range(2):
                bng = pc2.tile([128, NLT, 8], F32, tag="bng", name=f"bng{m}")
                nc.sync.dma_start(
                    out=bng,
                    in_=sc["bn_out"][m].rearrange("(lt p) c -> p lt c",
                                                  p=128))
                for lt in range(NLT):
                    ptb = ps("ptb")
                    ptbv = ptb[0:8, 0:128]
                    nc.tensor.transpose(ptbv, bng[:, lt, :], ident)
                    nc.scalar.mul(out=bnT[m][:, lt*128:(lt+1)*128],
                                  in_=ptbv, mul=1.0/d)

            # ---- hidden: stream back, bn-corr + gelu + logits, then the
            # logits ReduceScatter split per L-half so compute overlaps it.
            fb1_sb = pc1.tile([128, 8], F32)
            nc.sync.dma_start(out=fb1_sb,
                              in_=io["fb1"][:].rearrange("(m p) -> p m",
                                                         p=128))
            fw1b_sb = pc1.tile([8, 2, 1024], F16)
            nc.sync.dma_start(
                out=fw1b_sb,
                in_=io["fw1b"][:].rearrange("(m p) c -> p m c", p=8))
            fw2_sb = pc1.tile([128, 8, 12], F16)
            nc.sync.dma_start(
                out=fw2_sb,
                in_=io["fw2"][:].rearrange("(kt p) c -> p kt c", p=128))
            lg_sb = pc1.tile([128, NLT, 12], F32)
            for hf in range(2):
                for lw in (2*hf, 2*hf + 1):
                    hin = pc2.tile([128, 8, 512], F16, tag="hin", name="hin")
                    nc.sync.dma_start(
                        out=hin,
                        in_=sc["hdn_s"][:, lw*512:(lw+1)*512].rearrange(
                            "(mt p) l -> p mt l", p=128))
                    hfin = pc2.tile([128, 8, 512], F16, tag="hfin",
                                    name="hfin")
                    for mt in range(8):
                        pbn = ps("psbn")
                        for m in range(2):
                            nc.tensor.matmul(
                                pbn, fw1b_sb[:, m, mt*128:(mt+1)*128],
                                bnT[m][:, lw*512:(lw+1)*512],
                                start=(m == 0), stop=(m == 1))
                        htmp = pcs.tile([128, 512], F16, tag="htmp",
                                        name="htmp")
                        nc.vector.scalar_tensor_tensor(
                            out=htmp, in0=pbn, scalar=1.0,
                            in1=hin[:, mt, :], op0=ALU.mult, op1=ALU.add)
                        nc.scalar.activation(out=hfin[:, mt, :], in_=htmp,
                                             func=AF.Gelu,
                                             bias=fb1_sb[:, mt:mt+1])
                    for ltl in range(4):
                        lt = lw*4 + ltl
                        pl = ps("psl")
                        plv = pl[:, 0:12]
                        for kt in range(8):
                            nc.tensor.matmul(
                                plv, hfin[:, kt, ltl*128:(ltl+1)*128],
                                fw2_sb[:, kt, :],
                                start=(kt == 0), stop=(kt == 7))
                        nc.scalar.copy(out=lg_sb[:, lt, :], in_=plv)
                for m in range(2):
                    nc.sync.dma_start(
                        out=sc["lg_in"][hf, m].rearrange(
                            "(lt p) c -> p lt c", p=128),
                        in_=lg_sb[:, 8*hf:8*(hf+1), m*6:(m+1)*6])
                nc.gpsimd.collective_compute(
                    "ReduceScatter", ALU.add, replica_groups=RG,
                    ins=[sc["lg_in"][hf]],
                    outs=[sc["lg_out"][hf]])

            # ---- softmax gates (per L-half) ------------------------------
            b2_sb = pc1.tile([128, 6], F32)
            nc.sync.dma_start(out=b2_sb, in_=io["b2o"][:])
            lgo = pc1.tile([128, NLT, 2, 3], F32)
            rmax = pc1.tile([128, NLT, 2], F32)
            rsum = pc1.tile([128, NLT, 2], F32)

            def gates_half(hf):
                hsl = slice(8*hf, 8*(hf+1))
                lg = lgo[:, hsl]
                nc.sync.dma_start(
                    out=lg,
                    in_=sc["lg_out"][hf].rearrange(
                        "(lt p) (h e) -> p lt h e", p=128, e=3))
                nc.vector.tensor_tensor(
                    out=lg, in0=lg,
                    in1=bass.AP(tensor=b2_sb.tensor, offset=b2_sb.offset,
                                ap=[b2_sb.ap[0], [0, 8], [3, 2], [1, 3]]),
                    op=ALU.add)
                nc.vector.tensor_reduce(out=rmax[:, hsl], in_=lg,
                                        axis=mybir.AxisListType.X,
                                        op=ALU.max)
                nc.vector.tensor_tensor(
                    out=lg, in0=lg,
                    in1=rmax[:, hsl][:, :, :, None].to_broadcast(
                        [128, 8, 2, 3]),
                    op=ALU.subtract)
                nc.scalar.activation(out=lg, in_=lg, func=AF.Exp)
                nc.vector.tensor_reduce(out=rsum[:, hsl], in_=lg,
                                        axis=mybir.AxisListType.X,
                                        op=ALU.add)
                nc.vector.reciprocal(out=rsum[:, hsl], in_=rsum[:, hsl])
                nc.vector.tensor_tensor(
                    out=lg, in0=lg,
                    in1=rsum[:, hsl][:, :, :, None].to_broadcast(
                        [128, 8, 2, 3]),
                    op=ALU.mult)

            # ---- gate mix + RMSNorm + Wo (per L-half) --------------------
            wo_sb = pc1.tile([128, 4, D], F16)
            nc.sync.dma_start(
                out=wo_sb,
                in_=io["wo"][:].rearrange("(kt p) n -> p kt n", p=128))
            for hf in range(2):
                gates_half(hf)
                for lt in range(8*hf, 8*(hf+1)):
                    o_t = pcs.tile([128, 2, d], F16, tag="o_t", name="o_t")
                    ssq = pcs.tile([128, 2], F32, tag="ssq", name="ssq")
                    scr = pcs.tile([128, d], F32, tag="scr", name="scr")
                    for h in range(2):
                        nc.vector.tensor_scalar_mul(o_t[:, h, :],
                                                    cmv[:, lt, h, :],
                                                    lgo[:, lt, h, 0:1])
                        nc.vector.scalar_tensor_tensor(
                            out=o_t[:, h, :], in0=dov[:, lt, h, :],
                            scalar=lgo[:, lt, h, 1:2], in1=o_t[:, h, :],
                            op0=ALU.mult, op1=ALU.add)
                        nc.vector.scalar_tensor_tensor(
                            out=o_t[:, h, :], in0=vlc[:, lt, h, :],
                            scalar=lgo[:, lt, h, 2:3], in1=o_t[:, h, :],
                            op0=ALU.mult, op1=ALU.add)
                        nc.scalar.activation(out=scr, in_=o_t[:, h, :],
                                             func=AF.Square,
                                             accum_out=ssq[:, h:h+1])
                    nc.scalar.activation(out=ssq, in_=ssq, func=AF.Sqrt,
                                         scale=1.0/d, bias=eps5)
                    nc.vector.reciprocal(out=ssq, in_=ssq)
                    for h in range(2):
                        nc.vector.tensor_scalar_mul(o_t[:, h, :],
                                                    o_t[:, h, :],
                                                    ssq[:, h:h+1])
                    oT = pcs.tile([128, 4, 128], F16, tag="oT", name="oT")
                    for ct in range(4):
                        h, dt = ct // 2, ct % 2
                        pto = ps16("psto")
                        ptov = pto[:, 0:128]
                        nc.tensor.transpose(ptov,
                                            o_t[:, h, dt*128:(dt+1)*128],
                                            ident16)
                        cp(ct, oT[:, ct, :], ptov)
                    orow = pcs.tile([128, D], F32, tag="orow", name="orow")
                    for nh in range(2):
                        pw = ps("psw")
                        for ct in range(4):
                            nc.tensor.matmul(pw, oT[:, ct, :],
                                             wo_sb[:, ct,
                                                   nh*512:(nh+1)*512],
                                             start=(ct == 0), stop=(ct == 3))
                        cp(lt + nh, orow[:, nh*512:(nh+1)*512], pw)
                    nc.sync.dma_start(
                        out=io["out_part"][lt*128:(lt+1)*128, :], in_=orow)


# ======================= host side =======================================

def _diag_tiles(w_own, taps, out_dtype):
    """w_own: (C, k) conv weights for this core's channels.
    Returns (4, k, 128, 128) diag tiles; tap dd uses column k-1-dd."""
    k = w_own.shape[1]
    out = np.zeros((4, k, 128, 128), dtype=out_dtype)
    for ct in range(4):
        for dd in range(k):
            np.fill_diagonal(out[ct, dd], w_own[ct*128:(ct+1)*128, k-1-dd])
    return out


def _host_inputs(inputs):
    hs = np.asarray(inputs["hidden_states"], np.float32)
    Wq = np.asarray(inputs["Wq"], np.float32)
    Wk = np.asarray(inputs["Wk"], np.float32)
    Wv = np.asarray(inputs["Wv"], np.float32)
    Wb = np.asarray(inputs["Wb"], np.float32)
    cq = np.asarray(inputs["conv_q_w"], np.float32)
    ck = np.asarray(inputs["conv_k_w"], np.float32)
    cv = np.asarray(inputs["conv_v_w"], np.float32)
    w3 = np.asarray(inputs["ms_w3"], np.float32)
    w15 = np.asarray(inputs["ms_w15"], np.float32)
    w31 = np.asarray(inputs["ms_w31"], np.float32)
    kmix = np.asarray(inputs["kernel_mix_w"], np.float32)
    cmix = np.asarray(inputs["channel_mixer_w"], np.float32)
    fw1 = np.asarray(inputs["fusion_w1"], np.float32)
    fb1 = np.asarray(inputs["fusion_b1"], np.float32)
    fw2 = np.asarray(inputs["fusion_w2"], np.float32)
    fb2 = np.asarray(inputs["fusion_b2"], np.float32)
    onw = np.asarray(inputs["o_norm_w"], np.float32)
    Wo = np.asarray(inputs["Wo"], np.float32)

    # combined kernel_mix -> channel_mixer matrix Q: (3D, D)
    Q = np.zeros((3 * D, D), np.float32)
    for h in range(H):
        Q[h*3*d:(h+1)*3*d] = kmix @ cmix[h*d:(h+1)*d]

    masks = np.zeros((5, 128, 128), np.float32)
    i_, j_ = np.mgrid[0:128, 0:128]
    blk = (i_ // 32) == (j_ // 32)
    masks[0] = -((i_ > j_) & blk).astype(np.float32)
    masks[1] = -((i_ > j_) & ~blk).astype(np.float32)
    masks[2] = -((j_ > i_) & blk).astype(np.float32)
    masks[3] = (j_ >= i_).astype(np.float32)
    masks[4] = np.eye(128, dtype=np.float32)

    Wo_s = Wo * np.tile(onw, H)[:, None]

    in_maps = []
    for c in range(8):
        b, r = divmod(c, 2)
        cs = slice(C*r, C*(r+1))
        qmix = np.concatenate(
            [Q[1024*s + C*r: 1024*s + C*r + C] for s in range(3)], 0)
        # fp8 DoubleRow pairs: [ct, pair, i, p, f] diag tiles, taps padded
        msdiag = np.zeros((4, NPAIRS, 2, 128, 128), np.float32)
        base = 0
        for w, npr in zip((w3, w15, w31), MSP):
            ks = w.shape[1]
            wc = w[cs] * MS_SCALE
            for t in range(npr):
                for i in range(2):
                    dd = 2*t + (1 - i)   # i=0 holds tap 2t+1, i=1 tap 2t
                    if dd < ks:
                        for ct in range(4):
                            np.fill_diagonal(msdiag[ct, base+t, i],
                                             wc[ct*128:(ct+1)*128, ks-1-dd])
            base += npr
        msdiag = np.ascontiguousarray(
            msdiag.transpose(0, 3, 1, 2, 4)).astype(ml_dtypes.float8_e4m3)
        cdiag = np.stack([_diag_tiles(w[cs], KQKV, np.float16)
                          for w in (cq, ck, cv)], 0)
        cdiag = np.ascontiguousarray(cdiag.transpose(0, 3, 1, 2, 4))
        fw1b = np.zeros((16, 1024), np.float32)
        for m in range(2):
            for src in range(3):
                for h_ in range(2):
                    fw1b[m*8 + src*2 + h_] = \
                        fw1[D + src*4 + 2*m + h_, 1024*r:1024*(r+1)]
        fw2p = np.zeros((1024, 12), np.float32)
        b2o = np.zeros((6,), np.float32)
        for jm in range(2):
            for h_ in range(2):
                for br in range(3):
                    gcol = (2*jm + h_)*3 + br
                    fw2p[:, jm*6 + h_*3 + br] = fw2[1024*r:1024*(r+1), gcol]
        for h_ in range(2):
            for br in range(3):
                b2o[h_*3 + br] = fb2[(2*r + h_)*3 + br]
        m = {
            "hsT": np.ascontiguousarray(hs[b].T).astype(np.float16),
            "wq": np.ascontiguousarray(Wq[:, cs]).astype(np.float16),
            "wk": np.ascontiguousarray(Wk[:, cs]).astype(np.float16),
            "wv": np.ascontiguousarray(Wv[:, cs]).astype(np.float16),
            "wb": np.ascontiguousarray(Wb[:, 2*r:2*r+2]).astype(np.float16),
            "cdiag": cdiag,
            "msdiag": np.ascontiguousarray(msdiag),
            "qmix": (qmix * QMIX_SCALE).astype(ml_dtypes.float8_e4m3),
            "fw1h": np.ascontiguousarray(
                fw1[:D, 1024*r:1024*(r+1)]).astype(np.float16),
            "fw1b": fw1b.astype(np.float16),
            "fb1": np.ascontiguousarray(fb1[1024*r:1024*(r+1)]),
            "fw2": fw2p.astype(np.float16),
            "b2o": np.tile(b2o, (128, 1)),
            "wo": np.ascontiguousarray(Wo_s[cs, :]).astype(np.float16),
            "masks": masks,
            "onesrow": np.ones((1, 128), np.float32),
            "onescol": np.ones((128, 1), np.float32),
            "ident16": np.eye(128, dtype=np.float16),
        }
        in_maps.append(m)
    return in_maps


_PROG = {}


def _get_program(debug=False):
    key = bool(debug)
    if key not in _PROG:
        _PROG[key] = build_program(debug=debug)
    return _PROG[key]


def run(inputs, debug=False, **kw):
    nc = _get_program(debug=debug)
    in_maps = _host_inputs(inputs)
    res = run_bass_kernel_spmd(nc, in_maps, list(range(8)), **kw)
    return res


def kernel(**inputs):
    res = run(inputs)
    out = np.zeros((B, L, D), np.float32)
    for b in range(B):
        out[b] = res.results[2*b]["out_part"] + res.results[2*b+1]["out_part"]
    return out


if __name__ == "__main__":
    nc = build_program()
    print("program built ok")


# revision 34
# speedup vs baseline: 1.0739x; 1.0739x over previous
"""DeltaNet-style block (nn_DeltaNet_31877247271438) on 8 trn2 NeuronCores.

Sharding: core c -> (batch b = c//2, pair-rank r = c%2).  Within a batch pair:
  - head-parallel: rank r owns heads {2r, 2r+1} (feature cols [512r, 512r+512))
  - cross-head mixes are K-split with pairwise collectives:
      * channel_mixer (folded with kernel_mix into one matrix Q): partial sums
        ReduceScatter'ed (f16) so each core receives its own heads' ms_out
      * fusion-MLP hidden is column-split; logits partials ReduceScatter'ed
      * bn features AllGather'ed (tiny)
  - the final Wo matmul partials are summed on the host.

v2: single-pass schedule tuned for the TRN2 cost model:
  - all activations/projections stay SBUF-resident (no DRAM staging except
    the collectives and delta outputs); DMA count cut ~10x
  - delta rule runs as two 8-chunk batched halves (stage-major issue keeps
    the PE stream dense) followed by a short sequential scan over chunks;
    beta is folded in via per-partition scalar ops so no kb tensor exists
  - PSUM->SBUF copies round-robin over Act/DVE/Pool
  - fusion-MLP hidden (hs @ W1) is precomputed before the delta rule so only
    the small bn-correction + gelu remain after the AllGather
"""
import sys
sys.path.insert(0, '/opt/trn_rl_repo')

import numpy as np
import ml_dtypes

import concourse.bass as bass
import concourse.tile as tile
from concourse import bacc, mybir
from concourse.bass_utils import run_bass_kernel_spmd

F32 = mybir.dt.float32
BF16 = mybir.dt.bfloat16
F16 = mybir.dt.float16
F8 = mybir.dt.float8e4
AF = mybir.ActivationFunctionType
ALU = mybir.AluOpType

B, L, D, H = 4, 2048, 1024, 4
d = 256          # per-head dim
C = 512          # channels owned per core (2 heads)
NLT = 16         # l-tiles of 128
NLW = 4          # l-windows of 512
NCH = 16         # delta chunks of 128
KQKV = 4         # qkv conv taps
MSK = (3, 15, 31)
NTAPS = sum(MSK)  # 49
MSP = (2, 8, 16)  # fp8 DoubleRow tap-pairs per scale (taps padded to even)
NPAIRS = sum(MSP)  # 26
MS_SCALE = 32.0
QMIX_SCALE = 32.0
CM_DESCALE = 1.0 / (MS_SCALE * QMIX_SCALE)
PADV = 32
PAD = 4
RG = [[0, 1], [2, 3], [4, 5], [6, 7]]


def bc_mid(ap2, n):
    """[P, F] AP -> [P, n, F] with a 0-stride middle dim (free-dim bcast)."""
    assert len(ap2.ap) == 2
    return bass.AP(tensor=ap2.tensor, offset=ap2.offset,
                   ap=[ap2.ap[0], [0, n], ap2.ap[1]])


def build_program(debug=False):
    nc = bacc.Bacc("TRN2", target_bir_lowering=False, debug=False,
                   num_devices=8)

    io = {}
    io["hsT"] = nc.declare_dram_parameter("hsT", [D, L], F16, False)
    io["wq"] = nc.declare_dram_parameter("wq", [D, C], F16, False)
    io["wk"] = nc.declare_dram_parameter("wk", [D, C], F16, False)
    io["wv"] = nc.declare_dram_parameter("wv", [D, C], F16, False)
    io["wb"] = nc.declare_dram_parameter("wb", [D, 2], F16, False)
    io["cdiag"] = nc.declare_dram_parameter("cdiag", [3, 128, 4, KQKV, 128],
                                            F16, False)
    io["msdiag"] = nc.declare_dram_parameter("msdiag", [4, 128, NPAIRS, 2, 128],
                                             F8, False)
    io["qmix"] = nc.declare_dram_parameter("qmix", [12 * 128, D], F8, False)
    io["fw1h"] = nc.declare_dram_parameter("fw1h", [D, 1024], F16, False)
    io["fw1b"] = nc.declare_dram_parameter("fw1b", [16, 1024], F16, False)
    io["fb1"] = nc.declare_dram_parameter("fb1", [1024], F32, False)
    io["fw2"] = nc.declare_dram_parameter("fw2", [1024, 12], F16, False)
    io["b2o"] = nc.declare_dram_parameter("b2o", [128, 6], F32, False)
    io["wo"] = nc.declare_dram_parameter("wo", [C, D], F16, False)
    io["masks"] = nc.declare_dram_parameter("masks", [5, 128, 128], F32, False)
    io["onesrow"] = nc.declare_dram_parameter("onesrow", [1, 128], F32, False)
    io["onescol"] = nc.declare_dram_parameter("onescol", [128, 1], F32, False)
    io["ident16"] = nc.declare_dram_parameter("ident16", [128, 128], F16, False)
    io["out_part"] = nc.declare_dram_parameter("out_part", [L, D], F32, True)

    sc = {}
    sc["qT_s"] = nc.dram_tensor("qT_s", [2, 2, 128, L], F16)
    sc["hdn_s"] = nc.dram_tensor("hdn_s", [1024, L], F16)
    sc["cm_in"] = nc.dram_tensor("cm_in", [2, L, C], F16)
    sc["cm_out"] = nc.dram_tensor("cm_out", [L, C], F16)
    sc["dout_s"] = nc.dram_tensor("dout_s", [L, C], F16)
    sc["bn_in"] = nc.dram_tensor("bn_in", [L, 8], F32)
    sc["bn_out"] = nc.dram_tensor("bn_out", [2, L, 8], F32)
    sc["lg_in"] = nc.dram_tensor("lg_in", [2, 2, L // 2, 6], F32)
    sc["lg_out"] = nc.dram_tensor("lg_out", [2, L // 2, 6], F32)

    with tile.TileContext(nc) as tc:
        _body(nc, tc, io, sc)
    nc.compile()
    return nc


def _body(nc, tc, io, sc):
    from contextlib import ExitStack
    ctx = ExitStack()
    with ctx:
        consts = ctx.enter_context(tc.tile_pool(name="consts", bufs=1))
        glob = ctx.enter_context(tc.tile_pool(name="glob", bufs=1))

        masks = consts.tile([128, 5, 128], F32)
        nc.sync.dma_start(out=masks,
                          in_=io["masks"][:].rearrange("m p f -> p m f"))
        ident = masks[:, 4, :]
        ident16 = consts.tile([128, 128], F16)
        nc.sync.dma_start(out=ident16, in_=io["ident16"][:])
        onescol16 = consts.tile([128, 1], F16)
        nc.vector.memset(onescol16, 1.0)
        onesrow16 = consts.tile([1, 128], F16)
        nc.vector.memset(onesrow16, 1.0)
        eps6 = consts.tile([128, 1], F32)
        nc.vector.memset(eps6, 1e-6)
        eps5 = consts.tile([128, 1], F32)
        nc.vector.memset(eps5, 1e-5)

        beta_lp = consts.tile([128, NLT, 2], F32)
        # delta-rule state, ping-pong buffered across scan chunks
        S16 = consts.tile([128, 2, 2, 2, d], F16)  # (buf, h, kt, dv)
        nc.vector.memset(S16, 0.0)

        # persistent activations
        kT_sb = glob.tile([128, 2, 2, L], F16)     # k-hat, (h, dk-tile, l)
        klc = glob.tile([128, NLT, 2, d], F16)     # k-hat, l-major
        vlc = glob.tile([128, NLT, 2, d], F16)     # v, l-major
        bn_sb = glob.tile([128, NLT, 8], F32)
        nc.vector.memset(bn_sb, 0.0)

        hsT_r = io["hsT"][:].rearrange("(kt p) l -> p kt l", p=128)

        def cp(i, out, in_):
            # Pool/GPSIMD cannot touch PSUM; alternate Act and DVE
            if i % 2 == 0:
                nc.scalar.copy(out=out, in_=in_)
            else:
                nc.vector.tensor_copy(out=out, in_=in_)

        pvt_cm = tc.tile_pool(name="pvt", bufs=1)
        pvt = pvt_cm.__enter__()
        vt_bf = pvt.tile([128, 4, PADV + L], F16)  # conv-input v
        nc.vector.memset(vt_bf[:, :, 0:PADV], 0.0)
        v8_bf = pvt.tile([128, 4, PADV + L], F8)   # fp8 copy for ms-conv
        nc.gpsimd.memset(v8_bf[:, :, 0:PADV], 0.0)

        # =================== PHASE A ======================================
        with tc.tile_pool(name="pa1", bufs=1) as pa1, \
             tc.tile_pool(name="pa2", bufs=2) as pa2, \
             tc.tile_pool(name="pas", bufs=3) as pas, \
             tc.tile_pool(name="psa", bufs=4, space="PSUM") as psa, \
             tc.tile_pool(name="psb", bufs=2, space="PSUM") as psb:
            hsT = pa1.tile([128, 8, L], F16)
            nc.sync.dma_start(out=hsT, in_=hsT_r)

            def ps(name="pst", tag="pst"):
                return psa.tile([128, 512], F32, tag=tag, name=name)

            def ps16(name="pst16"):
                return psb.tile([128, 512], F16, tag="ps16", name=name)

            # ---- beta (row-form matmuls, then transpose to column) -------
            wb_sb = pa1.tile([128, 8, 2], F16)
            nc.sync.dma_start(
                out=wb_sb, in_=io["wb"][:].rearrange("(kt p) c -> p kt c",
                                                     p=128))
            for lw in range(NLW):
                pb = ps("psbeta")
                pbv = pb[0:2, :]
                for kt in range(8):
                    nc.tensor.matmul(pbv, wb_sb[:, kt, :],
                                     hsT[:, kt, lw*512:(lw+1)*512],
                                     start=(kt == 0), stop=(kt == 7))
                betar = pas.tile([2, 512], F32, tag="betar", bufs=2,
                                 name="betar")
                nc.scalar.activation(out=betar, in_=pbv, func=AF.Sigmoid)
                for ltl in range(4):
                    pt = ps("psbt")
                    ptv = pt[0:128, 0:2]
                    nc.tensor.transpose(ptv, betar[:, ltl*128:(ltl+1)*128],
                                        ident[0:2, 0:2])
                    nc.vector.tensor_copy(out=beta_lp[:, lw*4+ltl, :],
                                          in_=ptv)

            fw1h_sb = pa1.tile([128, 8, 1024], F16)
            nc.sync.dma_start(
                out=fw1h_sb,
                in_=io["fw1h"][:].rearrange("(kt p) m -> p kt m", p=128))

            # ---- q, k, v: proj -> conv -> silu ---------------------------
            # v first (feeds ms-conv + vlc); k silus straight into kT_sb
            # (normalized in place later); q stages through xc then DRAM.
            xc = None
            for tnm, ti in (("v", 2), ("q", 0), ("k", 1)):
                w_sb = pa2.tile([128, 8, C], F16, tag="w_sb", bufs=2,
                                name=f"w_{tnm}")
                nc.sync.dma_start(
                    out=w_sb,
                    in_=io["w" + tnm][:].rearrange("(kt p) c -> p kt c",
                                                   p=128))
                cdg = pa2.tile([128, 4, KQKV, 128], F16, tag="cdg", bufs=1,
                               name=f"cdg_{tnm}")
                nc.sync.dma_start(out=cdg, in_=io["cdiag"][ti])
                if tnm == "q":
                    xc = pa1.tile([128, 4, PAD + L], F16, name="xc_q")
                    nc.gpsimd.memset(xc[:, :, 0:PAD], 0.0)

                for ct in range(4):
                    xp = pa2.tile([128, PAD + L], F16, tag="xp", bufs=2,
                                  name=f"xp_{tnm}{ct}")
                    nc.gpsimd.memset(xp[:, 0:PAD], 0.0)
                    for lw in range(NLW):
                        pp = ps("psp")
                        for kt in range(8):
                            nc.tensor.matmul(
                                pp, w_sb[:, kt, ct*128:(ct+1)*128],
                                hsT[:, kt, lw*512:(lw+1)*512],
                                start=(kt == 0), stop=(kt == 7))
                        cp(ct + lw, xp[:, PAD+lw*512:PAD+(lw+1)*512], pp)
                    for lw in range(NLW):
                        pc = ps("psc")
                        for dd in range(KQKV):
                            off = PAD + lw*512 - dd
                            nc.tensor.matmul(
                                pc, cdg[:, ct, dd, :],
                                xp[:, off:off+512],
                                start=(dd == 0), stop=(dd == KQKV-1))
                        lsl = slice(lw*512, (lw+1)*512)
                        if tnm == "v":
                            nc.scalar.activation(
                                out=vt_bf[:, ct, PADV+lw*512:PADV+(lw+1)*512],
                                in_=pc, func=AF.Silu)
                        elif tnm == "k":
                            nc.scalar.activation(
                                out=kT_sb[:, ct // 2, ct % 2, lsl],
                                in_=pc, func=AF.Silu)
                        else:
                            nc.scalar.activation(
                                out=xc[:, ct, PAD+lw*512:PAD+(lw+1)*512],
                                in_=pc, func=AF.Silu)
                if tnm == "v":
                    # transpose v into l-major vlc
                    for ct in range(4):
                        h, dt = ct // 2, ct % 2
                        for lt in range(NLT):
                            ptr = ps16("psvt")
                            ptv = ptr[:, 0:128]
                            nc.tensor.transpose(
                                ptv,
                                vt_bf[:, ct, PADV+lt*128:PADV+(lt+1)*128],
                                ident16)
                            cp(lt % 3 % 2, vlc[:, lt, h, dt*128:(dt+1)*128], ptv)
                    for lt in range(NLT):
                        nc.vector.tensor_reduce(
                            out=bn_sb[:, lt, 4:6], in_=vlc[:, lt],
                            axis=mybir.AxisListType.X, op=ALU.add,
                            apply_absolute_value=True)
                    for ct in range(4):
                        nc.gpsimd.tensor_copy(out=v8_bf[:, ct, PADV:],
                                              in_=vt_bf[:, ct, PADV:])

            # ---- l2norm (q -> DRAM, k in place) + klc transposes,
            # interleaved with the fusion-MLP hidden matmuls so the PE
            # stream stays dense while the norm chains hop engines.
            filler = []

            def norm_item(tj, tnm, h, lw):
                def emit():
                    lsl = slice(lw*512, (lw+1)*512)

                    def src(dt):
                        if tnm == "q":
                            return xc[:, 2*h+dt, PAD+lsl.start:PAD+lsl.stop]
                        return kT_sb[:, h, dt, lsl]

                    sq = pas.tile([128, 2, 512], F16, tag="sq", bufs=2,
                                  name="sq")
                    pss = ps("psss")
                    pssv = pss[0:1, :]
                    for i in range(2):
                        nc.gpsimd.tensor_tensor(
                            out=sq[:, i, :], in0=src(i), in1=src(i),
                            op=ALU.mult)
                        nc.tensor.matmul(pssv, onescol16, sq[:, i, :],
                                         start=(i == 0), stop=(i == 1))
                    sr = pas.tile([1, 512], F32, tag="sr", bufs=2, name="sr")
                    nc.scalar.activation(out=sr, in_=pssv, func=AF.Sqrt,
                                         bias=eps6[0:1, :])
                    srt = pas.tile([1, 512], F16, tag="srt", bufs=2,
                                   name="srt")
                    with nc.allow_low_precision("l2norm scale fp16"):
                        nc.vector.reciprocal(out=srt, in_=sr)
                    pbc = ps("psbc")
                    nc.tensor.matmul(pbc, onesrow16, srt,
                                     start=True, stop=True)
                    for dt in range(2):
                        if tnm == "q":
                            qn = pas.tile([128, 512], F16, tag="qn",
                                          bufs=3, name="qn")
                            nc.vector.tensor_tensor(
                                out=qn, in0=src(dt), in1=pbc, op=ALU.mult)
                            nc.sync.dma_start(
                                out=sc["qT_s"][h, dt, :, lsl], in_=qn)
                        else:
                            nc.vector.tensor_tensor(
                                out=kT_sb[:, h, dt, lsl],
                                in0=src(dt), in1=pbc, op=ALU.mult)
                return emit

            def ktr_item(ct, lt0):
                def emit():
                    h, dt = ct // 2, ct % 2
                    for lt in range(lt0, lt0 + 8):
                        ptr = ps16("pskt")
                        ptv = ptr[:, 0:128]
                        nc.tensor.transpose(
                            ptv, kT_sb[:, h, dt, lt*128:(lt+1)*128],
                            ident16)
                        cp((lt + 1) % 3 % 2,
                           klc[:, lt, h, dt*128:(dt+1)*128], ptv)
                return emit

            for tnm, tj in (("q", 0), ("k", 1)):
                for h in range(2):
                    for lw in range(NLW):
                        filler.append(norm_item(tj, tnm, h, lw))
            for ct in range(4):
                for lt0 in (0, 8):
                    filler.append(ktr_item(ct, lt0))

            # fusion-MLP hidden partial (hs @ W1), pre-gelu -> DRAM
            for lw in range(NLW):
                hdl = pas.tile([128, 8, 512], F16, tag="hdl", bufs=2,
                               name="hdl")
                for mt in range(8):
                    ph = ps("psh")
                    for kt in range(8):
                        nc.tensor.matmul(ph,
                                         fw1h_sb[:, kt, mt*128:(mt+1)*128],
                                         hsT[:, kt, lw*512:(lw+1)*512],
                                         start=(kt == 0), stop=(kt == 7))
                    cp(lw + mt, hdl[:, mt, :], ph)
                    if filler:
                        filler.pop(0)()
                nc.sync.dma_start(
                    out=sc["hdn_s"][:, lw*512:(lw+1)*512].rearrange(
                        "(mt p) l -> p mt l", p=128),
                    in_=hdl)
            while filler:
                filler.pop(0)()

        # =================== PHASE B1/B2 (fused per l-window) =============
        # ms-conv + qmix run in fp8 DoubleRow: two shifted taps (or two
        # k-tiles) contract per instruction at 0.5 cycles/row.
        DR = mybir.MatmulPerfMode.DoubleRow
        with tc.tile_pool(name="pb1", bufs=1) as pb1, \
             tc.tile_pool(name="pbs", bufs=2) as pbs, \
             tc.tile_pool(name="psB", bufs=6, space="PSUM") as psB:
            qmix_sb = pb1.tile([128, 12, D], F8)
            nc.sync.dma_start(
                out=qmix_sb,
                in_=io["qmix"][:].rearrange("(kt p) o -> p kt o", p=128))
            ms8 = pb1.tile([128, 4, NPAIRS, 2, 128], F8)
            for ct in range(4):
                nc.sync.dma_start(out=ms8[:, ct], in_=io["msdiag"][ct])

            def pair_rhs(ct, off):
                # [128, 2, 512] with +1 stride: slice i starts at off-1+i,
                # i.e. i=0 -> tap 2t+1, i=1 -> tap 2t (host packs to match)
                a = v8_bf[:, ct, off-1:off-1+512]
                return bass.AP(tensor=a.tensor, offset=a.offset,
                               ap=[a.ap[0], [1, 2], a.ap[1]])

            for lw in range(NLW):
                y8 = pbs.tile([128, 12, 512], F8, tag="y", bufs=1, name="y")
                for ct in range(4):
                    base = 0
                    for si, np_ in enumerate(MSP):
                        py = psB.tile([128, 512], F32, tag="pB", name="psy")
                        for t in range(np_):
                            off = PADV + lw*512 - 2*t
                            nc.tensor.matmul(py, ms8[:, ct, base+t],
                                             pair_rhs(ct, off),
                                             start=(t == 0),
                                             stop=(t == np_-1),
                                             perf_mode=DR)
                        cp(ct + si, y8[:, si*4+ct, :], py)
                        base += np_
                cm_sb = pbs.tile([128, 4, 2, 512], F16, tag="cm",
                                 name=f"cm{lw}")
                for ltl in range(4):
                    for oh in range(2):
                        pq = psB.tile([128, 512], F32, tag="pB", name="psq")
                        for kp in range(6):
                            nc.tensor.matmul(
                                pq, y8[:, 2*kp:2*kp+2, ltl*128:(ltl+1)*128],
                                qmix_sb[:, 2*kp:2*kp+2, oh*512:(oh+1)*512],
                                start=(kp == 0), stop=(kp == 5),
                                perf_mode=DR)
                        if (ltl + oh) % 2 == 0:
                            nc.scalar.mul(out=cm_sb[:, ltl, oh, :], in_=pq,
                                          mul=CM_DESCALE)
                        else:
                            nc.vector.tensor_scalar_mul(
                                cm_sb[:, ltl, oh, :], pq, CM_DESCALE)
                for oh in range(2):
                    nc.sync.dma_start(
                        out=sc["cm_in"][oh, lw*512:(lw+1)*512, :].rearrange(
                            "(lt p) c -> p lt c", p=128),
                        in_=cm_sb[:, :, oh, :])
            nc.gpsimd.collective_compute(
                "ReduceScatter", ALU.add, replica_groups=RG,
                ins=[sc["cm_in"][:]], outs=[sc["cm_out"][:]])
        pvt_cm.__exit__(None, None, None)

        # =================== PHASE B3: delta rule =========================
        HC = 4                    # chunks per batch
        NB = NCH // HC            # number of batches
        NW = 2                    # 2-chunk waves per batch

        def bcm(m, n1=2, n2=2):
            """masks16[:, m, :] broadcast to [128, n1, n2, 128]."""
            ap = masks16[:, m, :]
            return bass.AP(tensor=ap.tensor, offset=ap.offset,
                           ap=[ap.ap[0], [0, n1], [0, n2], ap.ap[1]])

        ones128 = consts.tile([128, 128], F16)
        nc.vector.memset(ones128, 1.0)
        masks16 = consts.tile([128, 5, 128], F16)
        nc.vector.tensor_copy(out=masks16, in_=masks)

        with tc.tile_pool(name="pd1", bufs=1) as pd1, \
             tc.tile_pool(name="pdh", bufs=2) as pdh, \
             tc.tile_pool(name="pdo", bufs=3) as pdo, \
             tc.tile_pool(name="psD", bufs=4, space="PSUM") as psD, \
             tc.tile_pool(name="psU", bufs=3, space="PSUM") as psU:
            cmv = pd1.tile([128, NLT, 2, d], F16)

            def pd_ps(name):
                # one 2-chunk wave of 128-wide mats (2KB)
                return psD.tile([128, 2, 2, 128], F32, tag="pD", name=name)

            def pd16_ps(name, shape=None):
                return psD.tile(shape or [128, 2, 2, 128], F16, tag="pD",
                                name=name)

            def pw_ps(name):
                # one chunk of d-wide mats (2KB)
                return psD.tile([128, 2, d], F32, tag="pD", name=name)

            def pu_ps(name):
                return psU.tile([128, 2, d], F32, tag="pU", name=name)

            def hblk(tag, name=None):
                return pdh.tile([128, HC, 2, 128], F16, tag=tag,
                                name=name or tag)

            cp_i = [0]

            def bigcp(out, in_):
                cp_i[0] += 1
                cp(cp_i[0], out, in_)

            def psum_add(i, out, psum, addend):
                """out = psum + addend; odd i offloads to Act+Pool."""
                if i % 2 == 0:
                    nc.vector.scalar_tensor_tensor(
                        out=out, in0=psum, scalar=1.0, in1=addend,
                        op0=ALU.mult, op1=ALU.add)
                else:
                    nc.scalar.copy(out=out, in_=psum)
                    nc.gpsimd.tensor_tensor(out=out, in0=out, in1=addend,
                                            op=ALU.add)

            def scan_chain(qTb, u_all, wT_all, aT_all, j, ci):
                """The serial S-recurrence: minimal PE-queue footprint."""
                prev = ci % 2
                cur = 1 - prev
                pup = pu_ps(f"pup{ci}")
                for h in range(2):
                    for kt in range(2):
                        nc.tensor.matmul(pup[:, h, :],
                                         wT_all[:, j, kt, h, :],
                                         S16[:, prev, h, kt, :],
                                         start=(kt == 0), stop=(kt == 1))
                upr = pdo.tile([128, 2, d], F16, tag="upr", name="upr")
                nc.vector.scalar_tensor_tensor(
                    out=upr, in0=pup, scalar=-1.0, in1=u_all[:, j],
                    op0=ALU.mult, op1=ALU.add)
                for h in range(2):
                    pdS = pu_ps(f"pdS{ci}{h}")
                    for kt in range(2):
                        nc.tensor.matmul(pdS[:, kt, :],
                                         klc[:, ci, h, kt*128:(kt+1)*128],
                                         upr[:, h, :],
                                         start=True, stop=True)
                    nc.vector.scalar_tensor_tensor(
                        out=S16[:, cur, h], in0=pdS, scalar=1.0,
                        in1=S16[:, prev, h], op0=ALU.mult, op1=ALU.add)
                return upr

            def scan_out(qTb, aT_all, upr, j, ci):
                """Off-recurrence output: o_i, bn, dout staging."""
                prev = ci % 2
                po = pu_ps(f"po{ci}")
                for h in range(2):
                    for kt in range(2):
                        nc.tensor.matmul(po[:, h, :],
                                         qTb[:, h, kt, j*128:(j+1)*128],
                                         S16[:, prev, h, kt, :],
                                         start=(kt == 0), stop=False)
                    nc.tensor.matmul(po[:, h, :], aT_all[:, j, h, :],
                                     upr[:, h, :], start=False, stop=True)
                ot = pdo.tile([128, 2, d], F16, tag="ot", name="ot")
                nc.scalar.copy(out=ot, in_=po)
                nc.vector.tensor_reduce(
                    out=bn_sb[:, ci, 2:4], in_=ot,
                    axis=mybir.AxisListType.X, op=ALU.add,
                    apply_absolute_value=True)
                nc.sync.dma_start(
                    out=sc["dout_s"][ci*128:(ci+1)*128, :],
                    in_=ot.rearrange("p h e -> p (h e)"))

            def scan_chunk(qTb, u_all, wT_all, aT_all, j, ci):
                upr = scan_chain(qTb, u_all, wT_all, aT_all, j, ci)
                scan_out(qTb, aT_all, upr, j, ci)

            results = {}

            def batch_stages(bi):
                cs = list(range(bi*HC, (bi+1)*HC))
                qTb = pdh.tile([128, 2, 2, HC * 128], F16, tag="qTb",
                               bufs=4, name=f"qTb{bi}")
                nc.sync.dma_start(
                    out=qTb,
                    in_=sc["qT_s"][:, :, :,
                                   cs[0]*128:(cs[-1]+1)*128].rearrange(
                        "h kt p l -> p h kt l"))
                bB = pdh.tile([128, HC, 2, 128], F16, tag="tBB", name="bB")
                for j, ci in enumerate(cs):
                    for h in range(2):
                        nc.gpsimd.tensor_scalar_mul(
                            bB[:, j, h, :], ones128,
                            beta_lp[:, ci, h:h+1])
                yield

                Td, To, TdT = hblk("tA", "Td"), hblk("tTo", "To"), \
                    hblk("tB", "TdT")
                aT = pdh.tile([128, HC, 2, 128], F16, tag="taT", bufs=4,
                              name="aT")
                for wv in range(NW):
                    pg = pd_ps(f"pg{wv}")
                    pa_ = pd_ps(f"pa{wv}")
                    jsl = slice(wv*2, (wv+1)*2)
                    for jj in range(2):
                        j = wv*2 + jj
                        ci = cs[j]
                        for h in range(2):
                            for kt in range(2):
                                nc.tensor.matmul(
                                    pg[:, jj, h, :],
                                    kT_sb[:, h, kt, ci*128:(ci+1)*128],
                                    kT_sb[:, h, kt, ci*128:(ci+1)*128],
                                    start=(kt == 0), stop=(kt == 1))
                            for kt in range(2):
                                nc.tensor.matmul(
                                    pa_[:, jj, h, :],
                                    kT_sb[:, h, kt, ci*128:(ci+1)*128],
                                    qTb[:, h, kt, j*128:(j+1)*128],
                                    start=(kt == 0), stop=(kt == 1))
                    bG = pdo.tile([128, 2, 2, 128], F16, tag="bG",
                                  name=f"bG{wv}")
                    nc.vector.tensor_tensor(out=bG, in0=pg,
                                            in1=bB[:, jsl], op=ALU.mult)
                    nc.vector.tensor_tensor(out=Td[:, jsl], in0=bG,
                                              in1=bcm(0), op=ALU.mult)
                    nc.gpsimd.tensor_tensor(out=To[:, jsl], in0=bG,
                                            in1=bcm(1), op=ALU.mult)
                    nc.vector.tensor_tensor(out=aT[:, jsl], in0=pa_,
                                            in1=bcm(3), op=ALU.mult)
                    yield

                for wv in range(NW):
                    jsl = slice(wv*2, (wv+1)*2)
                    ptr = pd16_ps(f"ptt{wv}")
                    for jj in range(2):
                        j = wv*2 + jj
                        for h in range(2):
                            nc.tensor.transpose(ptr[:, jj, h, :],
                                                Td[:, j, h, :], ident16)
                    bigcp(TdT[:, jsl], ptr)
                yield
                MT = hblk("tE", "MT0")
                nc.vector.tensor_tensor(out=MT, in0=TdT,
                                        in1=bcm(4, n1=HC), op=ALU.add)
                lvl = {"Td": Td, "TdT": TdT}
                chain = [
                    ("T2", "TdT", "Td", "tC"), ("T2T", "Td", "TdT", "tD"),
                    ("T4", "T2T", "T2", "tA"), ("T4T", "T2", "T2T", "tB"),
                    ("T8", "T4T", "T4", "tC"), ("T8T", "T4", "T4T", "tD"),
                    ("T16", "T8T", "T8", "tA"),
                ]
                mt_tags = {"T2": "tF", "T4": "tE", "T8": "tF", "T16": "tG"}
                for si, (nm, ln, rn, tg) in enumerate(chain):
                    dst = hblk(tg, nm)
                    for wv in range(NW):
                        jsl = slice(wv*2, (wv+1)*2)
                        pq2 = pd_ps(f"pq_{nm}{wv}")
                        for jj in range(2):
                            j = wv*2 + jj
                            for h in range(2):
                                nc.tensor.matmul(
                                    pq2[:, jj, h, :], lvl[ln][:, j, h, :],
                                    lvl[rn][:, j, h, :],
                                    start=True, stop=True)
                        bigcp(dst[:, jsl], pq2)
                    lvl[nm] = dst
                    yield
                    if nm in mt_tags:
                        MTn = pdh.tile([128, HC, 2, 128], F16,
                                       tag=mt_tags[nm], name=f"MT_{nm}")
                        for wv in range(NW):
                            jsl = slice(wv*2, (wv+1)*2)
                            pm = pd_ps(f"pm_{nm}{wv}")
                            for jj in range(2):
                                j = wv*2 + jj
                                for h in range(2):
                                    nc.tensor.matmul(pm[:, jj, h, :],
                                                     lvl[nm][:, j, h, :],
                                                     MT[:, j, h, :],
                                                     start=True, stop=True)
                            psum_add(si + wv, MTn[:, jsl], pm, MT[:, jsl])
                        MT = MTn
                        yield
                DT = MT

                Bm, BT = hblk("tA", "Bm"), hblk("tB", "BT")
                for wv in range(NW):
                    jsl = slice(wv*2, (wv+1)*2)
                    pB = pd_ps(f"pB{wv}")
                    pBT = pd_ps(f"pBT{wv}")
                    for jj in range(2):
                        j = wv*2 + jj
                        for h in range(2):
                            nc.tensor.matmul(pB[:, jj, h, :],
                                             DT[:, j, h, :],
                                             To[:, j, h, :],
                                             start=True, stop=True)
                            nc.tensor.matmul(pBT[:, jj, h, :],
                                             To[:, j, h, :],
                                             DT[:, j, h, :],
                                             start=True, stop=True)
                    bigcp(Bm[:, jsl], pB)
                    bigcp(BT[:, jsl], pBT)
                yield
                B2T = hblk("tC", "B2T")
                for wv in range(NW):
                    jsl = slice(wv*2, (wv+1)*2)
                    pB2 = pd_ps(f"pB2{wv}")
                    for jj in range(2):
                        j = wv*2 + jj
                        for h in range(2):
                            nc.tensor.matmul(pB2[:, jj, h, :],
                                             Bm[:, j, h, :],
                                             BT[:, j, h, :],
                                             start=True, stop=True)
                    bigcp(B2T[:, jsl], pB2)
                yield

                u_all = pdh.tile([128, HC, 2, d], F16, tag="tU", name="u")
                w_all = pdh.tile([128, HC, 2, d], F16, tag="tW", name="w")
                for xi, (xnm, srcv, dstt) in enumerate(
                        (("u", vlc, u_all), ("w", klc, w_all))):
                    xb = pdh.tile([128, HC, 2, d], F16, tag="tXB", name="xb")
                    for j, ci in enumerate(cs):
                        for h in range(2):
                            nc.gpsimd.tensor_scalar_mul(
                                xb[:, j, h, :], srcv[:, ci, h, :],
                                beta_lp[:, ci, h:h+1])
                    x1 = pdh.tile([128, HC, 2, d], F16, tag="tX1", name="x1")
                    for j in range(HC):
                        px1 = pw_ps(f"px1{xnm}{j}")
                        for h in range(2):
                            nc.tensor.matmul(px1[:, h, :], DT[:, j, h, :],
                                             xb[:, j, h, :],
                                             start=True, stop=True)
                        bigcp(x1[:, j], px1)
                    yield
                    y1 = pdh.tile([128, HC, 2, d], F16, tag="tY1", name="y1")
                    for j in range(HC):
                        py1 = pw_ps(f"py1{xnm}{j}")
                        for h in range(2):
                            nc.tensor.matmul(py1[:, h, :], B2T[:, j, h, :],
                                             x1[:, j, h, :],
                                             start=True, stop=True)
                        psum_add(j + xi, y1[:, j], py1, x1[:, j])
                    yield
                    for j in range(HC):
                        pu = pw_ps(f"pu{xnm}{j}")
                        for h in range(2):
                            nc.tensor.matmul(pu[:, h, :], BT[:, j, h, :],
                                             y1[:, j, h, :],
                                             start=True, stop=True)
                        psum_add(j + xi + 1, dstt[:, j], pu, y1[:, j])
                    yield

                wT_all = pdh.tile([128, HC, 2, 2, 128], F16, tag="tWT",
                                  name="wT")
                for wv in range(NW):
                    ptw = pd16_ps(f"ptw{wv}", [128, 2, 2, 2, 128])
                    for jj in range(2):
                        j = wv*2 + jj
                        for kt in range(2):
                            for h in range(2):
                                nc.tensor.transpose(
                                    ptw[:, jj, kt, h, :],
                                    w_all[:, j, h, kt*128:(kt+1)*128],
                                    ident16)
                    bigcp(wT_all[:, wv*2:(wv+1)*2], ptw)
                yield
                results[bi] = (qTb, u_all, wT_all, aT)

            # drive batches pairwise; scans of the previous pair interleave
            for pair in range(NB // 2):
                if pair == 1:
                    # RS is done (or nearly) by now; safe to queue this load
                    nc.sync.dma_start(
                        out=cmv,
                        in_=sc["cm_out"][:].rearrange(
                            "(lt p) (h e) -> p lt h e", p=128, e=d))
                gens = [batch_stages(2*pair), batch_stages(2*pair + 1)]
                scan_q = ([(2*(pair-1)*HC + j) for j in range(2*HC)]
                          if pair > 0 else [])
                live = list(gens)
                rnd = 0
                pend = None   # (bsrc, upr, j, ci) waiting for scan_out
                while live:
                    for g in list(live):
                        try:
                            next(g)
                        except StopIteration:
                            live.remove(g)
                    rnd += 1
                    if pend is not None:
                        bsrc, upr, j, ci = pend
                        scan_out(bsrc[0], bsrc[3], upr, j, ci)
                        pend = None
                    elif scan_q and rnd % 2 == 0:
                        ci = scan_q.pop(0)
                        bsrc = results[ci // HC]
                        j = ci % HC
                        upr = scan_chain(bsrc[0], bsrc[1], bsrc[2],
                                         bsrc[3], j, ci)
                        pend = (bsrc, upr, j, ci)
                while scan_q or pend is not None:
                    if pend is not None:
                        bsrc, upr, j, ci = pend
                        scan_out(bsrc[0], bsrc[3], upr, j, ci)
                        pend = None
                    else:
                        ci = scan_q.pop(0)
                        bsrc = results[ci // HC]
                        j = ci % HC
                        upr = scan_chain(bsrc[0], bsrc[1], bsrc[2],
                                         bsrc[3], j, ci)
                        pend = (bsrc, upr, j, ci)
            for ci in range(2*HC, NCH):
                bsrc = results[ci // HC]
                scan_chunk(bsrc[0], bsrc[1], bsrc[2], bsrc[3],
                           ci % HC, ci)
                for lt in (2*(ci - 2*HC), 2*(ci - 2*HC) + 1):
                    nc.vector.tensor_reduce(
                        out=bn_sb[:, lt, 0:2], in_=cmv[:, lt],
                        axis=mybir.AxisListType.X, op=ALU.add,
                        apply_absolute_value=True)
            nc.sync.dma_start(
                out=sc["bn_in"][:].rearrange("(lt p) c -> p lt c", p=128),
                in_=bn_sb)
            nc.gpsimd.collective_compute(
                "AllGather", ALU.bypass, replica_groups=RG,
                ins=[sc["bn_in"][:]], outs=[sc["bn_out"][:]])

        # =================== PHASE C ======================================
        with tc.tile_pool(name="pc1", bufs=1) as pc1, \
             tc.tile_pool(name="pc2", bufs=2) as pc2, \
             tc.tile_pool(name="pcs", bufs=3) as pcs, \
             tc.tile_pool(name="psC", bufs=4, space="PSUM") as psC, \
             tc.tile_pool(name="psc2", bufs=2, space="PSUM") as psc2:

            def ps(name="pst"):
                return psC.tile([128, 512], F32, tag="pC", name=name)

            def ps16(name="pst16"):
                return psc2.tile([128, 512], F16, tag="pc16", name=name)

            # ---- bn features (cm reduced + AG'd at end of B3) ------------
            cmv = pc1.tile([128, NLT, 2, d], F16)
            nc.sync.dma_start(
                out=cmv,
                in_=sc["cm_out"][:].rearrange("(lt p) (h e) -> p lt h e",
                                              p=128, e=d))
            dov = pc1.tile([128, NLT, 2, d], F16)
            nc.sync.dma_start(
                out=dov,
                in_=sc["dout_s"][:].rearrange("(lt p) (h e) -> p lt h e",
                                              p=128, e=d))

            bnT = [pc1.tile([8, L], F16, name=f"bnT{m}") for m in range(2)]
            for m in range(2):
                bng = pc2.tile([128, NLT, 8], F32, tag="bng", name=f"bng{m}")
                nc.sync.dma_start(
                    out=bng,
                    in_=sc["bn_out"][m].rearrange("(lt p) c -> p lt c",
                                                  p=128))
                for lt in range(NLT):
                    ptb = ps("ptb")
                    ptbv = ptb[0:8, 0:128]
                    nc.tensor.transpose(ptbv, bng[:, lt, :], ident)
                    nc.scalar.mul(out=bnT[m][:, lt*128:(lt+1)*128],
                                  in_=ptbv, mul=1.0/d)

            # ---- hidden: stream back, bn-corr + gelu + logits, then the
            # logits ReduceScatter split per L-half so compute overlaps it.
            fb1_sb = pc1.tile([128, 8], F32)
            nc.sync.dma_start(out=fb1_sb,
                              in_=io["fb1"][:].rearrange("(m p) -> p m",
                                                         p=128))
            fw1b_sb = pc1.tile([8, 2, 1024], F16)
            nc.sync.dma_start(
                out=fw1b_sb,
                in_=io["fw1b"][:].rearrange("(m p) c -> p m c", p=8))
            fw2_sb = pc1.tile([128, 8, 12], F16)
            nc.sync.dma_start(
                out=fw2_sb,
                in_=io["fw2"][:].rearrange("(kt p) c -> p kt c", p=128))
            lg_sb = pc1.tile([128, NLT, 12], F32)
            for hf in range(2):
                for lw in (2*hf, 2*hf + 1):
                    hin = pc2.tile([128, 8, 512], F16, tag="hin", name="hin")
                    nc.sync.dma_start(
                        out=hin,
                        in_=sc["hdn_s"][:, lw*512:(lw+1)*512].rearrange(
                            "(mt p) l -> p mt l", p=128))
                    hfin = pc2.tile([128, 8, 512], F16, tag="hfin",
                                    name="hfin")
                    for mt in range(8):
                        pbn = ps("psbn")
                        for m in range(2):
                            nc.tensor.matmul(
                                pbn, fw1b_sb[:, m, mt*128:(mt+1)*128],
                                bnT[m][:, lw*512:(lw+1)*512],
                                start=(m == 0), stop=(m == 1))
                        htmp = pcs.tile([128, 512], F16, tag="htmp",
                                        name="htmp")
                        nc.vector.scalar_tensor_tensor(
                            out=htmp, in0=pbn, scalar=1.0,
                            in1=hin[:, mt, :], op0=ALU.mult, op1=ALU.add)
                        nc.scalar.activation(out=hfin[:, mt, :], in_=htmp,
                                             func=AF.Gelu,
                                             bias=fb1_sb[:, mt:mt+1])
                    for ltl in range(4):
                        lt = lw*4 + ltl
                        pl = ps("psl")
                        plv = pl[:, 0:12]
                        for kt in range(8):
                            nc.tensor.matmul(
                                plv, hfin[:, kt, ltl*128:(ltl+1)*128],
                                fw2_sb[:, kt, :],
                                start=(kt == 0), stop=(kt == 7))
                        nc.scalar.copy(out=lg_sb[:, lt, :], in_=plv)
                for m in range(2):
                    nc.sync.dma_start(
                        out=sc["lg_in"][hf, m].rearrange(
                            "(lt p) c -> p lt c", p=128),
                        in_=lg_sb[:, 8*hf:8*(hf+1), m*6:(m+1)*6])
                nc.gpsimd.collective_compute(
                    "ReduceScatter", ALU.add, replica_groups=RG,
                    ins=[sc["lg_in"][hf]],
                    outs=[sc["lg_out"][hf]])

            # ---- softmax gates (per L-half) ------------------------------
            b2_sb = pc1.tile([128, 6], F32)
            nc.sync.dma_start(out=b2_sb, in_=io["b2o"][:])
            lgo = pc1.tile([128, NLT, 2, 3], F32)
            rmax = pc1.tile([128, NLT, 2], F32)
            rsum = pc1.tile([128, NLT, 2], F32)

            def gates_half(hf):
                hsl = slice(8*hf, 8*(hf+1))
                lg = lgo[:, hsl]
                nc.sync.dma_start(
                    out=lg,
                    in_=sc["lg_out"][hf].rearrange(
                        "(lt p) (h e) -> p lt h e", p=128, e=3))
                nc.vector.tensor_tensor(
                    out=lg, in0=lg,
                    in1=bass.AP(tensor=b2_sb.tensor, offset=b2_sb.offset,
                                ap=[b2_sb.ap[0], [0, 8], [3, 2], [1, 3]]),
                    op=ALU.add)
                nc.vector.tensor_reduce(out=rmax[:, hsl], in_=lg,
                                        axis=mybir.AxisListType.X,
                                        op=ALU.max)
                nc.vector.tensor_tensor(
                    out=lg, in0=lg,
                    in1=rmax[:, hsl][:, :, :, None].to_broadcast(
                        [128, 8, 2, 3]),
                    op=ALU.subtract)
                nc.scalar.activation(out=lg, in_=lg, func=AF.Exp)
                nc.vector.tensor_reduce(out=rsum[:, hsl], in_=lg,
                                        axis=mybir.AxisListType.X,
                                        op=ALU.add)
                nc.vector.reciprocal(out=rsum[:, hsl], in_=rsum[:, hsl])
                nc.vector.tensor_tensor(
                    out=lg, in0=lg,
                    in1=rsum[:, hsl][:, :, :, None].to_broadcast(
                        [128, 8, 2, 3]),
                    op=ALU.mult)

            # ---- gate mix + RMSNorm + Wo (per L-half) --------------------
            wo_sb = pc1.tile([128, 4, D], F16)
            nc.sync.dma_start(
                out=wo_sb,
                in_=io["wo"][:].rearrange("(kt p) n -> p kt n", p=128))
            for hf in range(2):
                gates_half(hf)
                for lt in range(8*hf, 8*(hf+1)):
                    o_t = pcs.tile([128, 2, d], F16, tag="o_t", name="o_t")
                    ssq = pcs.tile([128, 2], F32, tag="ssq", name="ssq")
                    scr = pcs.tile([128, d], F32, tag="scr", name="scr")
                    for h in range(2):
                        nc.vector.tensor_scalar_mul(o_t[:, h, :],
                                                    cmv[:, lt, h, :],
                                                    lgo[:, lt, h, 0:1])
                        nc.vector.scalar_tensor_tensor(
                            out=o_t[:, h, :], in0=dov[:, lt, h, :],
                            scalar=lgo[:, lt, h, 1:2], in1=o_t[:, h, :],
                            op0=ALU.mult, op1=ALU.add)
                        nc.vector.scalar_tensor_tensor(
                            out=o_t[:, h, :], in0=vlc[:, lt, h, :],
                            scalar=lgo[:, lt, h, 2:3], in1=o_t[:, h, :],
                            op0=ALU.mult, op1=ALU.add)
                        nc.scalar.activation(out=scr, in_=o_t[:, h, :],
                                             func=AF.Square,
                                             accum_out=ssq[:, h:h+1])
                    nc.scalar.activation(out=ssq, in_=ssq, func=AF.Sqrt,
                                         scale=1.0/d, bias=eps5)
                    nc.vector.reciprocal(out=ssq, in_=ssq)
                    for h in range(2):
                        nc.vector.tensor_scalar_mul(o_t[:, h, :],
                                                    o_t[:, h, :],
                                                    ssq[:, h:h+1])
                    oT = pcs.tile([128, 4, 128], F16, tag="oT", name="oT")
                    for ct in range(4):
                        h, dt = ct // 2, ct % 2
                        pto = ps16("psto")
                        ptov = pto[:, 0:128]
                        nc.tensor.transpose(ptov,
                                            o_t[:, h, dt*128:(dt+1)*128],
                                            ident16)
                        cp(ct, oT[:, ct, :], ptov)
                    orow = pcs.tile([128, D], F32, tag="orow", name="orow")
                    for nh in range(2):
                        pw = ps("psw")
                        for ct in range(4):
                            nc.tensor.matmul(pw, oT[:, ct, :],
                                             wo_sb[:, ct,
                                                   nh*512:(nh+1)*512],
                                             start=(ct == 0), stop=(ct == 3))
                        cp(lt + nh, orow[:, nh*512:(nh+1)*512], pw)
                    nc.sync.dma_start(
                        out=io["out_part"][lt*128:(lt+1)*128, :], in_=orow)


# ======================= host side =======================================

def _diag_tiles(w_own, taps, out_dtype):
    """w_own: (C, k) conv weights for this core's channels.
    Returns (4, k, 128, 128) diag tiles; tap dd uses column k-1-dd."""
    k = w_own.shape[1]
    out = np.zeros((4, k, 128, 128), dtype=out_dtype)
    for ct in range(4):
        for dd in range(k):
            np.fill_diagonal(out[ct, dd], w_own[ct*128:(ct+1)*128, k-1-dd])
    return out


def _host_inputs(inputs):
    hs = np.asarray(inputs["hidden_states"], np.float32)
    Wq = np.asarray(inputs["Wq"], np.float32)
    Wk = np.asarray(inputs["Wk"], np.float32)
    Wv = np.asarray(inputs["Wv"], np.float32)
    Wb = np.asarray(inputs["Wb"], np.float32)
    cq = np.asarray(inputs["conv_q_w"], np.float32)
    ck = np.asarray(inputs["conv_k_w"], np.float32)
    cv = np.asarray(inputs["conv_v_w"], np.float32)
    w3 = np.asarray(inputs["ms_w3"], np.float32)
    w15 = np.asarray(inputs["ms_w15"], np.float32)
    w31 = np.asarray(inputs["ms_w31"], np.float32)
    kmix = np.asarray(inputs["kernel_mix_w"], np.float32)
    cmix = np.asarray(inputs["channel_mixer_w"], np.float32)
    fw1 = np.asarray(inputs["fusion_w1"], np.float32)
    fb1 = np.asarray(inputs["fusion_b1"], np.float32)
    fw2 = np.asarray(inputs["fusion_w2"], np.float32)
    fb2 = np.asarray(inputs["fusion_b2"], np.float32)
    onw = np.asarray(inputs["o_norm_w"], np.float32)
    Wo = np.asarray(inputs["Wo"], np.float32)

    # combined kernel_mix -> channel_mixer matrix Q: (3D, D)
    Q = np.zeros((3 * D, D), np.float32)
    for h in range(H):
        Q[h*3*d:(h+1)*3*d] = kmix @ cmix[h*d:(h+1)*d]

    masks = np.zeros((5, 128, 128), np.float32)
    i_, j_ = np.mgrid[0:128, 0:128]
    blk = (i_ // 32) == (j_ // 32)
    masks[0] = -((i_ > j_) & blk).astype(np.float32)
    masks[1] = -((i_ > j_) & ~blk).astype(np.float32)
    masks[2] = -((j_ > i_) & blk).astype(np.float32)
    masks[3] = (j_ >= i_).astype(np.float32)
    masks[4] = np.eye(128, dtype=np.float32)

    Wo_s = Wo * np.tile(onw, H)[:, None]

    in_maps = []
    for c in range(8):
        b, r = divmod(c, 2)
        cs = slice(C*r, C*(r+1))
        qmix = np.concatenate(
            [Q[1024*s + C*r: 1024*s + C*r + C] for s in range(3)], 0)
        # fp8 DoubleRow pairs: [ct, pair, i, p, f] diag tiles, taps padded
        msdiag = np.zeros((4, NPAIRS, 2, 128, 128), np.float32)
        base = 0
        for w, npr in zip((w3, w15, w31), MSP):
            ks = w.shape[1]
            wc = w[cs] * MS_SCALE
            for t in range(npr):
                for i in range(2):
                    dd = 2*t + (1 - i)   # i=0 holds tap 2t+1, i=1 tap 2t
                    if dd < ks:
                        for ct in range(4):
                            np.fill_diagonal(msdiag[ct, base+t, i],
                                             wc[ct*128:(ct+1)*128, ks-1-dd])
            base += npr
        msdiag = np.ascontiguousarray(
            msdiag.transpose(0, 3, 1, 2, 4)).astype(ml_dtypes.float8_e4m3)
        cdiag = np.stack([_diag_tiles(w[cs], KQKV, np.float16)
                          for w in (cq, ck, cv)], 0)
        cdiag = np.ascontiguousarray(cdiag.transpose(0, 3, 1, 2, 4))
        fw1b = np.zeros((16, 1024), np.float32)
        for m in range(2):
            for src in range(3):
                for h_ in range(2):
                    fw1b[m*8 + src*2 + h_] = \
                        fw1[D + src*4 + 2*m + h_, 1024*r:1024*(r+1)]
        fw2p = np.zeros((1024, 12), np.float32)
        b2o = np.zeros((6,), np.float32)
        for jm in range(2):
            for h_ in range(2):
                for br in range(3):
                    gcol = (2*jm + h_)*3 + br
                    fw2p[:, jm*6 + h_*3 + br] = fw2[1024*r:1024*(r+1), gcol]
        for h_ in range(2):
            for br in range(3):
                b2o[h_*3 + br] = fb2[(2*r + h_)*3 + br]
        m = {
            "hsT": np.ascontiguousarray(hs[b].T).astype(np.float16),
            "wq": np.ascontiguousarray(Wq[:, cs]).astype(np.float16),
            "wk": np.ascontiguousarray(Wk[:, cs]).astype(np.float16),
            "wv": np.ascontiguousarray(Wv[:, cs]).astype(np.float16),
            "wb": np.ascontiguousarray(Wb[:, 2*r:2*r+2]).astype(np.float16),
            "cdiag": cdiag,
            "msdiag": np.ascontiguousarray(msdiag),
            "qmix": (qmix * QMIX_SCALE).astype(ml_dtypes.float8_e4m3),
            "fw1h": np.ascontiguousarray(
                fw1[:D, 1024*r:1024*(r+1)]).astype(np.float16),
            "fw1b": fw1b.astype(np.float16),
            "fb1": np.ascontiguousarray(fb1[1024*r:1024*(r+1)]),
            "fw2": fw2p.astype(np.float16),
            "b2o": np.tile(b2o, (128, 1)),
            "wo": np.ascontiguousarray(Wo_s[cs, :]).astype(np.float16),
            "masks": masks,
            "onesrow": np.ones((1, 128), np.float32),
            "onescol": np.ones((128, 1), np.float32),
            "ident16": np.eye(128, dtype=np.float16),
        }
        in_maps.append(m)
    return in_maps


_PROG = {}


def _get_program(debug=False):
    key = bool(debug)
    if key not in _PROG:
        _PROG[key] = build_program(debug=debug)
    return _PROG[key]


def run(inputs, debug=False, **kw):
    nc = _get_program(debug=debug)
    in_maps = _host_inputs(inputs)
    res = run_bass_kernel_spmd(nc, in_maps, list(range(8)), **kw)
    return res


def kernel(**inputs):
    res = run(inputs)
    out = np.zeros((B, L, D), np.float32)
    for b in range(B):
        out[b] = res.results[2*b]["out_part"] + res.results[2*b+1]["out_part"]
    return out


if __name__ == "__main__":
    nc = build_program()
    print("program built ok")
